# revision 1
# baseline (speedup 1.0000x reference)
"""Bidirectional Mamba block (in_proj -> depthwise causal conv -> SiLU ->
forward+backward S6 selective scan -> gated combine -> out_proj) as a
Trainium2 Bass/Tile SPMD kernel over 8 NeuronCores.

Sharding: tensor-parallel over d_inner (256 channels per core). The conv and
the S6 scans are channel-independent, so they need no communication. Two
small collectives:
  * AllReduce of the partial x-projection dbc = u @ Wx^T (contraction over
    all d_inner) per direction  (~768KB each)
  * ReduceScatter of the partial out-projection (each core ends with its
    token shard of the final output; the host concatenates the 8 shards).

Scan layout: partitions = (8 channels x 16 states), free dim = L.  The S6
recurrence h[t] = dA[t]*h[t-1] + dBu[t] runs on the DVE hardware scan
instruction (tensor_tensor_scan, fp32 internal state).  The backward
direction reuses the same pipeline with reversed free-dim access patterns on
the scan only.  dA = exp(A * delta_rep) is produced by the scalar engine
directly from PSUM (PE replicates delta across the 16 state partitions via a
tiny selection matmul, ACT applies exp with the per-partition scale A).
"""

import os
import sys

for _p in ("/opt/trn_rl_repo", "/root/.axon_site/_ro/trn_rl_repo"):
    if os.path.isdir(_p) and _p not in sys.path:
        sys.path.append(_p)

from dataclasses import dataclass

import ml_dtypes
import numpy as np

import concourse.bass as bass
import concourse.mybir as mybir
import concourse.tile as tile
from concourse import bacc

DT = mybir.dt.float32
F32R = mybir.dt.float32r
BF = mybir.dt.bfloat16
AF = mybir.ActivationFunctionType
OP = mybir.AluOpType


@dataclass(frozen=True)
class Cfg:
    n_cores: int = 8
    B: int = 2
    L: int = 1024
    M: int = 1024      # d_model
    DI: int = 2048     # d_inner
    N: int = 16        # d_state
    R: int = 64        # dt_rank
    KC: int = 4        # conv kernel

    @property
    def DC(self):  # channels per core
        return self.DI // self.n_cores

    @property
    def TOK(self):
        return self.B * self.L

    @property
    def P_CH(self):  # partitions per channel tile
        return min(128, self.DC)

    @property
    def CHT(self):  # channel tiles per core
        return self.DC // self.P_CH

    @property
    def NT(self):  # scan tiles per (dir, batch): 8 channels each
        return self.DC // 8

    @property
    def TPC(self):  # scan tiles per channel tile
        return self.P_CH // 8

    @property
    def FCH(self):  # matmul moving-dim chunk over tokens (never spans batches)
        return min(512, self.L)

    @property
    def E(self):
        return self.R + 2 * self.N

    def check(self):
        assert self.DC % 8 == 0 and self.DC % self.P_CH == 0
        assert self.M % 128 == 0
        assert self.TOK % 128 == 0 and self.TOK % self.FCH == 0
        assert self.L % min(512, self.L) == 0
        assert self.N == 16


FULL = Cfg()


def build_consts(cfg: Cfg):
    """Selection matrices used as PE 'weights' (exact 0/1 values).

    All matmul moving operands must start at base partition 0, so row
    selection/replication is folded into the stationary matrix.
    """
    P = 128
    ident = np.eye(P, dtype=np.float32)
    # R_all[:, jj, :]: out[p] = src[8*jj + p//16]  (delta/w replication)
    r_all = np.zeros((cfg.P_CH, cfg.TPC, P), np.float32)
    for jj in range(cfg.TPC):
        for p in range(P):
            r_all[8 * jj + p // 16, jj, p] = 1.0
    # T_sel[:, which, :]: out[p] = src[16*which + p%16]  (B/C replication)
    t_sel = np.zeros((2 * cfg.N, 2, P), np.float32)
    for which in range(2):
        for p in range(P):
            t_sel[cfg.N * which + p % 16, which, p] = 1.0
    # S_all[:, jj, :]: reduce groups of 16 partitions into channel 8*jj+p//16
    s_all = np.zeros((P, cfg.TPC, cfg.P_CH), np.float32)
    for jj in range(cfg.TPC):
        for p in range(P):
            s_all[p, jj, 8 * jj + p // 16] = 1.0
    return ident, r_all, t_sel, s_all


def build_program(cfg: Cfg) -> bass.Bass:
    cfg.check()
    P = 128
    TOK, L, M = cfg.TOK, cfg.L, cfg.M
    DC, CHT, P_CH, NT, TPC, FCH = (cfg.DC, cfg.CHT, cfg.P_CH, cfg.NT,
                                   cfg.TPC, cfg.FCH)
    MT = M // P               # m tiles
    TBT = TOK // P            # token blocks
    NFC = TOK // FCH          # token chunks
    E, R, N = cfg.E, cfg.R, cfg.N
    LH = min(512, L)          # matmul chunk within one sequence
    NLH = L // LH

    nc = bacc.Bacc(
        "TRN2", target_bir_lowering=False, debug=False, num_devices=cfg.n_cores
    )

    # ---- kernel I/O ----
    x_d = nc.dram_tensor("x", [TOK, M], DT, kind="ExternalInput")
    winuT_d = nc.dram_tensor("winuT", [M, DC], F32R, kind="ExternalInput")
    winrT_d = nc.dram_tensor("winrT", [M, DC], F32R, kind="ExternalInput")
    wconv_d = nc.dram_tensor("wconv", [P, CHT * cfg.KC], DT, kind="ExternalInput")
    bconv_d = nc.dram_tensor("bconv", [P, CHT], DT, kind="ExternalInput")
    wxT_d = {d: nc.dram_tensor(f"wx{d}T", [DC, E], F32R, kind="ExternalInput")
             for d in "fb"}
    wdtT_d = {d: nc.dram_tensor(f"wdt{d}T", [R, DC], F32R, kind="ExternalInput")
              for d in "fb"}
    bdt_d = {d: nc.dram_tensor(f"bdt{d}", [P, CHT], DT, kind="ExternalInput")
             for d in "fb"}
    acol_d = {d: nc.dram_tensor(f"acol{d}", [P, NT], DT, kind="ExternalInput")
              for d in "fb"}
    dsum_d = nc.dram_tensor("dsum", [P, CHT], DT, kind="ExternalInput")
    woutT_d = nc.dram_tensor("woutT", [DC, M], F32R, kind="ExternalInput")
    ident_d = nc.dram_tensor("ident", [P, P], DT, kind="ExternalInput")
    rall_d = nc.dram_tensor("rall", [P_CH, TPC * P], BF, kind="ExternalInput")
    tsel_d = nc.dram_tensor("tsel", [2 * N, 2 * P], F32R, kind="ExternalInput")
    sall_d = nc.dram_tensor("sall", [P, TPC * P_CH], BF, kind="ExternalInput")

    out_d = nc.dram_tensor("out_rs", [TOK // cfg.n_cores, M], DT,
                           kind="ExternalOutput")

    rg = [list(range(cfg.n_cores))]
    cc_space = "Shared" if cfg.n_cores > 4 else "Local"

    with tile.TileContext(nc) as tc:
        with tc.tile_pool(name="persist", bufs=1) as pp, \
             tc.tile_pool(name="dram", bufs=1, space="DRAM") as dp:

            # ---------- persistent SBUF (small weights + gate activations) --
            ident_s = pp.tile([P, P], DT)
            nc.sync.dma_start(ident_s[:], ident_d.ap())
            rall_s = pp.tile([P_CH, TPC, P], BF)
            nc.sync.dma_start(rall_s[:], rall_d.ap().rearrange(
                "k (a b) -> k a b", a=TPC))
            tsel_s = pp.tile([2 * N, 2, P], F32R)
            nc.sync.dma_start(tsel_s[:], tsel_d.ap().rearrange(
                "k (a b) -> k a b", a=2))
            sall_s = pp.tile([P, TPC, P_CH], BF)
            nc.sync.dma_start(sall_s[:], sall_d.ap().rearrange(
                "p (a b) -> p a b", a=TPC))
            wconv_s = pp.tile([P, CHT, cfg.KC], DT)
            nc.sync.dma_start(wconv_s[:], wconv_d.ap().rearrange(
                "p (c k) -> p c k", c=CHT))
            bconv_s = pp.tile([P, CHT], DT)
            nc.sync.dma_start(bconv_s[:], bconv_d.ap())
            wx_s, wdt_s, bdt_s, acol_s = {}, {}, {}, {}
            for d in "fb":
                wx_s[d] = pp.tile([P_CH, CHT, E], F32R, name=f"wx{d}_s")
                nc.sync.dma_start(wx_s[d][:], wxT_d[d].ap().rearrange(
                    "(c p) e -> p c e", p=P_CH))
                wdt_s[d] = pp.tile([R, DC], F32R, name=f"wdt{d}_s")
                nc.sync.dma_start(wdt_s[d][:], wdtT_d[d].ap())
                bdt_s[d] = pp.tile([P, CHT], DT, name=f"bdt{d}_s")
                nc.sync.dma_start(bdt_s[d][:], bdt_d[d].ap())
                acol_s[d] = pp.tile([P, NT], DT, name=f"acol{d}_s")
                nc.sync.dma_start(acol_s[d][:], acol_d[d].ap())
            dsum_s = pp.tile([P, CHT], DT)
            nc.sync.dma_start(dsum_s[:], dsum_d.ap())
            wout_s = pp.tile([P_CH, CHT, M], F32R)
            nc.sync.dma_start(wout_s[:], woutT_d.ap().rearrange(
                "(c p) m -> p c m", p=P_CH))

            u_c = [pp.tile([P_CH, TOK], F32R, name=f"u_c{c}") for c in range(CHT)]
            sres = [pp.tile([P_CH, TOK], DT, name=f"sres{c}")
                    for c in range(CHT)]

            # ---------- phase 0-2: x^T, in_proj, conv, silu ----------
            with tc.tile_pool(name="proj", bufs=1) as jp, \
                 tc.tile_pool(name="proj_ps", bufs=1, space="PSUM") as jpp:
                xT = [jp.tile([P, TOK], F32R, name=f"xT{mt}") for mt in range(MT)]
                win_s = jp.tile([P, MT, 2 * DC], F32R)
                nc.sync.dma_start(win_s[:, :, :DC], winuT_d.ap().rearrange(
                    "(a p) c -> p a c", p=P))
                nc.sync.dma_start(win_s[:, :, DC:], winrT_d.ap().rearrange(
                    "(a p) c -> p a c", p=P))

                TPG = min(4, MT)  # transposes grouped per PSUM tile
                for tb in range(TBT):
                    xsb = jp.tile([P, M], DT, tag="xsb", bufs=2, name="xsb")
                    nc.sync.dma_start(xsb[:], x_d.ap()[tb * P:(tb + 1) * P, :])
                    for mg in range(MT // TPG):
                        tp_ps = jpp.tile([P, TPG * P], DT, tag="tp", bufs=4,
                                         name="tp_ps")
                        for k in range(TPG):
                            mt = mg * TPG + k
                            nc.tensor.transpose(
                                tp_ps[:, k * P:(k + 1) * P],
                                xsb[:, mt * P:(mt + 1) * P], ident_s[:])
                        for k in range(TPG):
                            mt = mg * TPG + k
                            nc.vector.tensor_copy(
                                xT[mt][:, tb * P:(tb + 1) * P],
                                tp_ps[:, k * P:(k + 1) * P])

                # padded conv inputs (filled by in_proj PSUM evacuation)
                upad = [[jp.tile([P_CH, cfg.KC - 1 + L], DT,
                                 name=f"upad{c}_{b}")
                         for b in range(cfg.B)] for c in range(CHT)]
                for c in range(CHT):
                    for b in range(cfg.B):
                        nc.gpsimd.memset(upad[c][b][:, :cfg.KC - 1], 0.0)

                for c in range(CHT):
                    for fc in range(NFC):
                        f0 = fc * FCH
                        ups = jpp.tile([P_CH, FCH], DT, tag="mm", bufs=4,
                                       name="ups")
                        for kt in range(MT):
                            nc.tensor.matmul(
                                ups[:],
                                win_s[:, kt, c * P_CH:(c + 1) * P_CH]
                                ,
                                xT[kt][:, f0:f0 + FCH],
                                start=(kt == 0), stop=(kt == MT - 1))
                        b = f0 // L
                        off = f0 % L
                        nc.scalar.copy(
                            upad[c][b][:, cfg.KC - 1 + off:
                                       cfg.KC - 1 + off + FCH], ups[:])

                # depthwise causal conv + SiLU
                with tc.tile_pool(name="conv", bufs=1) as cp:
                    for c in range(CHT):
                        for b in range(cfg.B):
                            acc = None
                            for k in range(cfg.KC):
                                nxt = cp.tile([P_CH, L], DT, tag="cacc",
                                              bufs=2, name="cacc")
                                tap = upad[c][b][:, k:k + L]
                                wk = wconv_s[:P_CH, c, k:k + 1]
                                if acc is None:
                                    nc.vector.tensor_scalar(
                                        nxt[:], tap, wk,
                                        bconv_s[:P_CH, c:c + 1],
                                        OP.mult, OP.add)
                                else:
                                    nc.vector.scalar_tensor_tensor(
                                        nxt[:], tap, wk, acc[:],
                                        OP.mult, OP.add)
                                acc = nxt
                            sg2 = cp.tile([P_CH, L], DT, tag="sg2", bufs=2,
                                          name="sg2")
                            nc.scalar.activation(sg2[:], acc[:], AF.Sigmoid)
                            nc.gpsimd.tensor_tensor(
                                u_c[c][:, b * L:(b + 1) * L], acc[:], sg2[:],
                                OP.mult)

                # ------ phase 3: dbc partials + AllReduce; the res
                # projection is emitted between the two directions so it
                # overlaps the first AllReduce's network time ------
                dbc_part = {d: dp.tile([E, TOK], DT, name=f"dbc_part_{d}")
                            for d in "fb"}
                dbc_red = {d: dp.tile([E, TOK], DT, addr_space=cc_space,
                                      name=f"dbc_red_{d}") for d in "fb"}

                def dbc_dir(d):
                    for fc in range(NFC):
                        f0 = fc * FCH
                        bps = jpp.tile([E, FCH], DT, tag="mm", bufs=4,
                                       name="bps")
                        for c in range(CHT):
                            nc.tensor.matmul(
                                bps[:],
                                wx_s[d][:, c, :],
                                u_c[c][:, f0:f0 + FCH],
                                start=(c == 0), stop=(c == CHT - 1))
                        bst = jp.tile([E, FCH], DT, tag="bst", bufs=3,
                                      name="bst")
                        nc.scalar.copy(bst[:], bps[:])
                        nc.sync.dma_start(dbc_part[d][:, f0:f0 + FCH], bst[:])
                    nc.gpsimd.collective_compute(
                        "AllReduce", OP.add, replica_groups=rg,
                        ins=[dbc_part[d].opt()], outs=[dbc_red[d].opt()])

                dbc_dir("f")
                for c in range(CHT):
                    for fc in range(NFC):
                        f0 = fc * FCH
                        rps = jpp.tile([P_CH, FCH], DT, tag="mm", bufs=4,
                                       name="rps")
                        for kt in range(MT):
                            nc.tensor.matmul(
                                rps[:],
                                win_s[:, kt, DC + c * P_CH:DC + (c + 1) * P_CH],
                                xT[kt][:, f0:f0 + FCH],
                                start=(kt == 0), stop=(kt == MT - 1))
                        sg = jp.tile([P_CH, FCH], DT, tag="sg", bufs=2,
                                     name="sg")
                        nc.scalar.activation(sg[:], rps[:], AF.Sigmoid)
                        nc.vector.tensor_tensor(sres[c][:, f0:f0 + FCH],
                                                rps[:], sg[:], OP.mult)
                dbc_dir("b")

            # ---------- phase 4: per-direction delta prep + scan ----------
            # Scan tiles are batch-merged [128, TOK]: one scan instruction
            # spans both batch segments; dA at each later segment's first
            # (in scan order) element is zeroed so no state leaks across.
            y_f = [pp.tile([P_CH, TOK], F32R, name=f"y_f{c}") for c in range(CHT)]

            with tc.tile_pool(name="scan_sb", bufs=1) as sp, \
                 tc.tile_pool(name="scan_ps", bufs=1, space="PSUM") as spp, \
                 tc.tile_pool(name="comb", bufs=1) as kp:
                for d in "fb":
                    # dt/BC from the reduced projection
                    dt_sb = sp.tile([R, TOK], F32R, tag="dt", bufs=1,
                                    name=f"dt_{d}")
                    nc.sync.dma_start(dt_sb[:], dbc_red[d][:R, :].bitcast(F32R))
                    bc_sb = sp.tile([2 * N, TOK], F32R, tag="bc", bufs=1,
                                    name=f"bc_{d}")
                    nc.sync.dma_start(bc_sb[:], dbc_red[d][R:, :].bitcast(F32R))

                    # B/C replicated across the 8-channel groups, full TOK
                    brep = sp.tile([P, TOK], BF, tag="brep", bufs=2,
                                   name=f"brep{d}")
                    crep = sp.tile([P, TOK], BF, tag="crep", bufs=2,
                                   name=f"crep{d}")
                    for which, rep in ((0, brep), (1, crep)):
                        for lh in range(TOK // LH):
                            o = lh * LH
                            rps2 = spp.tile([P, LH], DT, tag="rep",
                                            bufs=2, name="rps2")
                            nc.tensor.matmul(
                                rps2[:],
                                tsel_s[:, which, :],
                                bc_sb[:, o:o + LH],
                                start=True, stop=True)
                            nc.scalar.copy(rep[:, o:o + LH], rps2[:])

                    # delta = softplus(dt @ WdtT + bdt) [bf16]; w = delta * u
                    delta = [sp.tile([P_CH, TOK], BF, tag=f"delta{c}", bufs=2,
                                     name=f"delta_{d}{c}") for c in range(CHT)]
                    w_s = [sp.tile([P_CH, TOK], BF, tag=f"w{c}", bufs=2,
                                   name=f"w_{d}{c}") for c in range(CHT)]
                    for c in range(CHT):
                        for fc in range(NFC):
                            f0 = fc * FCH
                            dps = spp.tile([P_CH, FCH], DT, tag="rep", bufs=2,
                                           name="dps")
                            nc.tensor.matmul(
                                dps[:],
                                wdt_s[d][:, c * P_CH:(c + 1) * P_CH],
                                dt_sb[:, f0:f0 + FCH],
                                start=True, stop=True)
                            # softplus(x + bdt) = ln(1 + exp(x + bdt))
                            spt = sp.tile([P_CH, FCH], DT, tag="spt", bufs=1,
                                          name="spt")
                            nc.scalar.activation(
                                spt[:], dps[:], AF.Exp,
                                bias=bdt_s[d][:P_CH, c:c + 1])
                            nc.scalar.activation(
                                delta[c][:, f0:f0 + FCH], spt[:], AF.Ln,
                                bias=1.0)
                        nc.vector.tensor_tensor(
                            w_s[c][:], delta[c][:], u_c[c][:], OP.mult)

                    for j in range(NT):
                        c = j // TPC
                        jj = j % TPC
                        rsel = rall_s[:, jj, :]
                        dA = sp.tile([P, TOK], DT, tag="dA", bufs=2,
                                     name="dA")
                        dBu = sp.tile([P, TOK], DT, tag="dBu", bufs=2,
                                      name="dBu")
                        for b in range(cfg.B):
                            o = b * L
                            dp_ps = spp.tile([P, L], DT, tag="rep", bufs=2,
                                             name="dp_ps")
                            for lh in range(NLH):
                                q = lh * LH
                                nc.tensor.matmul(
                                    dp_ps[:, q:q + LH], rsel,
                                    delta[c][:, o + q:o + q + LH],
                                    start=True, stop=True)
                            nc.scalar.activation(
                                dA[:, o:o + L], dp_ps[:], AF.Exp,
                                scale=acol_s[d][:, j:j + 1])
                            w_ps = spp.tile([P, L], DT, tag="rep", bufs=2,
                                            name="w_ps")
                            for lh in range(NLH):
                                q = lh * LH
                                nc.tensor.matmul(
                                    w_ps[:, q:q + LH], rsel,
                                    w_s[c][:, o + q:o + q + LH],
                                    start=True, stop=True)
                            nc.vector.tensor_tensor(
                                dBu[:, o:o + L], w_ps[:],
                                brep[:, o:o + L], OP.mult)
                        # kill cross-batch state leakage at the segment
                        # boundary in scan order
                        if d == "f":
                            nc.gpsimd.memset(dA[:, L:L + 1], 0.0)
                        else:
                            nc.gpsimd.memset(dA[:, L - 1:L], 0.0)
                        h = sp.tile([P, TOK], DT, tag="h", bufs=2, name="h")
                        if d == "f":
                            nc.vector.tensor_tensor_scan(
                                h[:], dA[:], dBu[:], 0.0, OP.mult, OP.add)
                        else:
                            nc.vector.tensor_tensor_scan(
                                h[:, ::-1], dA[:, ::-1], dBu[:, ::-1],
                                0.0, OP.mult, OP.add)
                        hC = sp.tile([P, TOK], BF, tag="hC", bufs=2,
                                     name="hC")
                        nc.gpsimd.tensor_tensor(hC[:], h[:], crep[:], OP.mult)
                        if jj == 0:
                            y_ps = [spp.tile([P_CH, L], DT, tag=f"y{b}",
                                             bufs=1, name=f"y_ps{b}")
                                    for b in range(cfg.B)]
                        for b in range(cfg.B):
                            for lh in range(NLH):
                                q = lh * LH
                                nc.tensor.matmul(
                                    y_ps[b][:, q:q + LH],
                                    sall_s[:, jj, :],
                                    hC[:, b * L + q:b * L + q + LH],
                                    start=(jj == 0), stop=(jj == TPC - 1))
                        if jj != TPC - 1:
                            continue
                        for b in range(cfg.B):
                            ysl = y_f[c][:, b * L:(b + 1) * L]
                            if d == "f":
                                nc.scalar.copy(ysl, y_ps[b][:])
                            else:
                                # fused combine:
                                # y = (y_f + y_b + u*(fD+bD)) * (0.5*silu(res))
                                # (the 0.5 is folded into W_out host-side)
                                t1 = kp.tile([P_CH, L], DT, tag="t5", bufs=2,
                                             name="t1")
                                nc.vector.tensor_tensor(t1[:], y_ps[b][:],
                                                        ysl, OP.add)
                                t2 = kp.tile([P_CH, L], DT, tag="t5", bufs=2,
                                             name="t2")
                                nc.vector.scalar_tensor_tensor(
                                    t2[:], u_c[c][:, b * L:(b + 1) * L],
                                    dsum_s[:P_CH, c:c + 1], t1[:],
                                    OP.mult, OP.add)
                                nc.vector.tensor_tensor(
                                    ysl, t2[:], sres[c][:, b * L:(b + 1) * L],
                                    OP.mult)

            # ---------- phase 6: out_proj + ReduceScatter ----------
            out_part = dp.tile([TOK, M], DT, name="out_part")
            out_rs = dp.tile([TOK // cfg.n_cores, M], DT,
                             name="out_rs_b")
            with tc.tile_pool(name="out_ps", bufs=1, space="PSUM") as opp, \
                 tc.tile_pool(name="out_sb", bufs=1) as osp:
                MFC = min(512, M)
                for tb in range(TBT):
                    ops = opp.tile([P, M], DT, tag="out", bufs=2, name="ops")
                    for mc in range(M // MFC):
                        o = mc * MFC
                        for c in range(CHT):
                            nc.tensor.matmul(
                                ops[:, o:o + MFC],
                                y_f[c][:, tb * P:(tb + 1) * P],
                                wout_s[:, c, o:o + MFC],
                                start=(c == 0), stop=(c == CHT - 1))
                    ost = osp.tile([P, M], DT, tag="ost", bufs=2, name="ost")
                    nc.scalar.copy(ost[:], ops[:])
                    nc.sync.dma_start(out_part[tb * P:(tb + 1) * P, :],
                                      ost[:])
            nc.gpsimd.collective_compute(
                "ReduceScatter", OP.add, replica_groups=rg,
                ins=[out_part.opt()], outs=[out_rs.opt()])
            nc.sync.dma_start(out_d.ap(), out_rs[:])

    nc.compile()
    return nc


# --------------------------------------------------------------------------
# host side
# --------------------------------------------------------------------------

def host_prep(cfg: Cfg, inputs: dict) -> list[dict]:
    """Slice the full-model inputs into one input map per core."""
    P = 128
    f32 = np.float32

    def g(name):
        return np.asarray(inputs[name], f32)

    x = g("x").reshape(cfg.TOK, cfg.M)
    W_in = g("W_in")
    W_conv = g("W_conv").reshape(cfg.DI, cfg.KC)
    b_conv = g("b_conv")
    W_out = g("W_out")
    ident, r_all, t_sel, s_all = build_consts(cfg)
    sall_flat = s_all.reshape(P, cfg.TPC * cfg.P_CH)
    rall_flat = r_all.reshape(cfg.P_CH, cfg.TPC * P)
    tsel_flat = t_sel.reshape(2 * cfg.N, 2 * P)

    per = {}
    for d in "fb":
        per[d] = dict(
            A=-np.exp(g(d + "A_log")),            # (DI, N)
            D=g(d + "D"),
            Wx=g(d + "Wx"),                       # (E, DI)
            Wdt=g(d + "Wdt"),                     # (DI, R)
            bdt=g(d + "bdt"),
        )

    def col_layout(v):  # (DC,) -> (P_CH, CHT): [p, c] = v[c*P_CH + p]
        return np.ascontiguousarray(
            v.reshape(cfg.CHT, cfg.P_CH).T.astype(f32))

    def pad_p(a):  # pad partition dim up to 128
        if a.shape[0] == P:
            return np.ascontiguousarray(a.astype(f32))
        out = np.zeros((P,) + a.shape[1:], f32)
        out[:a.shape[0]] = a
        return out

    in_maps = []
    for core in range(cfg.n_cores):
        c0 = core * cfg.DC
        ch = slice(c0, c0 + cfg.DC)
        m = {
            "x": x,
            "winuT": np.ascontiguousarray(W_in[ch, :].T),
            "winrT": np.ascontiguousarray(
                W_in[cfg.DI + c0:cfg.DI + c0 + cfg.DC, :].T),
            "wconv": pad_p(
                W_conv[ch].reshape(cfg.CHT, cfg.P_CH, cfg.KC)
                .transpose(1, 0, 2).reshape(cfg.P_CH, cfg.CHT * cfg.KC)),
            "bconv": pad_p(col_layout(b_conv[ch])),
            "dsum": pad_p(col_layout(per["f"]["D"][ch] + per["b"]["D"][ch])),
            "woutT": np.ascontiguousarray(W_out[:, ch].T * 0.5),
            "ident": ident,
            "rall": rall_flat.astype(ml_dtypes.bfloat16),
            "tsel": tsel_flat,
            "sall": sall_flat.astype(ml_dtypes.bfloat16),
        }
        for d in "fb":
            pd = per[d]
            m[f"wx{d}T"] = np.ascontiguousarray(pd["Wx"][:, ch].T)
            m[f"wdt{d}T"] = np.ascontiguousarray(pd["Wdt"][ch, :].T)
            m[f"bdt{d}"] = pad_p(col_layout(pd["bdt"][ch]))
            # A columns: [p, j] = A[8j + p//16, p%16] (local channels)
            Ac = pd["A"][ch]                       # (DC, N)
            acol = np.empty((P, cfg.NT), f32)
            pidx = np.arange(P)
            for j in range(cfg.NT):
                acol[:, j] = Ac[8 * j + pidx // 16, pidx % 16]
            m[f"acol{d}"] = acol
        in_maps.append({k: np.ascontiguousarray(v) for k, v in m.items()})
    return in_maps


def gather_out(cfg: Cfg, results: list[dict]) -> np.ndarray:
    shards = [np.asarray(results[i]["out_rs"]) for i in range(cfg.n_cores)]
    out = np.concatenate(shards, axis=0)
    return out.reshape(cfg.B, cfg.L, cfg.M).astype(np.float32)


def kernel(**inputs) -> np.ndarray:
    cfg = FULL
    from concourse.bass_utils import run_bass_kernel_spmd
    nc = build_program(cfg)
    in_maps = host_prep(cfg, inputs)
    res = run_bass_kernel_spmd(nc, in_maps, core_ids=list(range(cfg.n_cores)))
    return gather_out(cfg, res.results)



# revision 4
# speedup vs baseline: 1.1316x; 1.1316x over previous
"""Bidirectional Mamba block (in_proj -> depthwise causal conv -> SiLU ->
forward+backward S6 selective scan -> gated combine -> out_proj) as a
Trainium2 Bass/Tile SPMD kernel over 8 NeuronCores.

Sharding: tensor-parallel over d_inner (256 channels per core). The conv and
the S6 scans are channel-independent, so they need no communication. Two
small collectives:
  * AllReduce (bf16) of the partial x-projection dbc = u @ Wx^T per direction
  * Chunked ReduceScatter of the partial out-projection, overlapped with the
    out_proj matmuls; the host reassembles the 8 shards.

Compute dtypes: bf16 operands everywhere (fp32 PSUM accumulation), which
doubles/quadruples DVE elementwise throughput and halves DMA traffic. The S6
recurrence runs on the DVE tensor_tensor_scan (fp32 internal state).
Activation-table usage is phase-ordered (Silu early, Exp/Ln for the scan
phase) to avoid ACT_TABLE_LOAD thrash.
"""

import os
import sys

for _p in ("/opt/trn_rl_repo", "/root/.axon_site/_ro/trn_rl_repo"):
    if os.path.isdir(_p) and _p not in sys.path:
        sys.path.append(_p)

from dataclasses import dataclass

import ml_dtypes
import numpy as np

import concourse.bass as bass
import concourse.mybir as mybir
import concourse.tile as tile
from concourse import bacc

DT = mybir.dt.float32
BF = mybir.dt.bfloat16
AF = mybir.ActivationFunctionType
OP = mybir.AluOpType


@dataclass(frozen=True)
class Cfg:
    n_cores: int = 8
    B: int = 2
    L: int = 1024
    M: int = 1024      # d_model
    DI: int = 2048     # d_inner
    N: int = 16        # d_state
    R: int = 64        # dt_rank
    KC: int = 4        # conv kernel
    RSC: int = 4       # ReduceScatter chunks

    @property
    def DC(self):  # channels per core
        return self.DI // self.n_cores

    @property
    def TOK(self):
        return self.B * self.L

    @property
    def P_CH(self):  # partitions per channel tile
        return min(128, self.DC)

    @property
    def CHT(self):  # channel tiles per core
        return self.DC // self.P_CH

    @property
    def NT(self):  # scan tiles per (dir, batch): 8 channels each
        return self.DC // 8

    @property
    def TPC(self):  # scan tiles per channel tile
        return self.P_CH // 8

    @property
    def FCH(self):  # matmul moving-dim chunk over tokens (never spans batches)
        return min(512, self.L)

    @property
    def E(self):
        return self.R + 2 * self.N

    def check(self):
        assert self.DC % 8 == 0 and self.DC % self.P_CH == 0
        assert self.M % 128 == 0
        assert self.TOK % 128 == 0 and self.TOK % self.FCH == 0
        assert self.L % min(512, self.L) == 0
        assert self.N == 16
        assert self.TOK % (self.RSC * self.n_cores) == 0


FULL = Cfg()


def build_consts(cfg: Cfg):
    """Selection matrices used as PE 'weights' (exact 0/1 values)."""
    P = 128
    ident = np.eye(P, dtype=np.float32)
    # R_all[:, jj, :]: out[p] = src[8*jj + p//16]  (delta/w replication)
    r_all = np.zeros((cfg.P_CH, cfg.TPC, P), np.float32)
    for jj in range(cfg.TPC):
        for p in range(P):
            r_all[8 * jj + p // 16, jj, p] = 1.0
    # T_sel[:, which, :]: out[p] = src[16*which + p%16]  (B/C replication)
    t_sel = np.zeros((2 * cfg.N, 2, P), np.float32)
    for which in range(2):
        for p in range(P):
            t_sel[cfg.N * which + p % 16, which, p] = 1.0
    # S_all[:, jj, :]: reduce groups of 16 partitions into channel 8*jj+p//16
    s_all = np.zeros((P, cfg.TPC, cfg.P_CH), np.float32)
    for jj in range(cfg.TPC):
        for p in range(P):
            s_all[p, jj, 8 * jj + p // 16] = 1.0
    return ident, r_all, t_sel, s_all


def build_program(cfg: Cfg) -> bass.Bass:
    cfg.check()
    P = 128
    TOK, L, M = cfg.TOK, cfg.L, cfg.M
    DC, CHT, P_CH, NT, TPC, FCH = (cfg.DC, cfg.CHT, cfg.P_CH, cfg.NT,
                                   cfg.TPC, cfg.FCH)
    MT = M // P               # m tiles
    TBT = TOK // P            # token blocks
    NFC = TOK // FCH          # token chunks
    E, R, N = cfg.E, cfg.R, cfg.N
    LH = min(512, L)          # matmul chunk within one sequence
    NLH = L // LH

    nc = bacc.Bacc(
        "TRN2", target_bir_lowering=False, debug=False, num_devices=cfg.n_cores
    )

    # ---- kernel I/O ----
    x_d = nc.dram_tensor("x", [TOK, M], BF, kind="ExternalInput")
    winuT_d = nc.dram_tensor("winuT", [M, DC], BF, kind="ExternalInput")
    winrT_d = nc.dram_tensor("winrT", [M, DC], BF, kind="ExternalInput")
    wconv_d = nc.dram_tensor("wconv", [P, CHT * cfg.KC], DT, kind="ExternalInput")
    bconv_d = nc.dram_tensor("bconv", [P, CHT], DT, kind="ExternalInput")
    wxT_d = {d: nc.dram_tensor(f"wx{d}T", [DC, E], BF, kind="ExternalInput")
             for d in "fb"}
    wdtT_d = {d: nc.dram_tensor(f"wdt{d}T", [R, DC], BF, kind="ExternalInput")
              for d in "fb"}
    bdt_d = {d: nc.dram_tensor(f"bdt{d}", [P, CHT], DT, kind="ExternalInput")
             for d in "fb"}
    acol_d = {d: nc.dram_tensor(f"acol{d}", [P, NT], DT, kind="ExternalInput")
              for d in "fb"}
    dsum_d = nc.dram_tensor("dsum", [P, CHT], DT, kind="ExternalInput")
    woutT_d = nc.dram_tensor("woutT", [DC, M], BF, kind="ExternalInput")
    ident_d = nc.dram_tensor("ident", [P, P], BF, kind="ExternalInput")
    rall_d = nc.dram_tensor("rall", [P_CH, TPC * P], BF, kind="ExternalInput")
    tsel_d = nc.dram_tensor("tsel", [2 * N, 2 * P], BF, kind="ExternalInput")
    sall_d = nc.dram_tensor("sall", [P, TPC * P_CH], BF, kind="ExternalInput")

    RSC = cfg.RSC
    RCH = TOK // RSC                    # rows per RS chunk
    RSH = RCH // cfg.n_cores            # rows per core per RS chunk
    out_d = nc.dram_tensor("out_rs", [TOK // cfg.n_cores, M], DT,
                           kind="ExternalOutput")
    DBG = os.environ.get("KDBG", "") == "1"
    dbg = {}
    if DBG:
        for nm, shp, dt_ in [("d_u0", [P_CH, TOK], BF),
                             ("d_sres0", [P_CH, TOK], BF),
                             ("d_delta0", [P_CH, TOK], BF),
                             ("d_dA0", [P, TOK], BF),
                             ("d_dBu0", [P, TOK], BF),
                             ("d_h0", [P, TOK], BF),
                             ("d_yf0", [P_CH, TOK], BF),
                             ("d_xT0", [P, TOK], BF)]:
            dbg[nm] = nc.dram_tensor(nm, shp, dt_, kind="ExternalOutput")
    dbc_dbg = {d: nc.dram_tensor(f"d_dbc_{d}", [E, TOK], BF,
                                 kind="ExternalOutput") if DBG else None
               for d in "fb"}

    rg = [list(range(cfg.n_cores))]
    cc_space = "Shared" if cfg.n_cores > 4 else "Local"

    with tile.TileContext(nc) as tc:
        with tc.tile_pool(name="persist", bufs=1) as pp, \
             tc.tile_pool(name="dram", bufs=1, space="DRAM") as dp:

            # ---------- persistent SBUF (small weights + gate activations) --
            ident_s = pp.tile([P, P], BF)
            nc.sync.dma_start(ident_s[:], ident_d.ap())
            rall_s = pp.tile([P_CH, TPC, P], BF)
            nc.sync.dma_start(rall_s[:], rall_d.ap().rearrange(
                "k (a b) -> k a b", a=TPC))
            tsel_s = pp.tile([2 * N, 2, P], BF)
            nc.sync.dma_start(tsel_s[:], tsel_d.ap().rearrange(
                "k (a b) -> k a b", a=2))
            sall_s = pp.tile([P, TPC, P_CH], BF)
            nc.sync.dma_start(sall_s[:], sall_d.ap().rearrange(
                "p (a b) -> p a b", a=TPC))
            wconv_s = pp.tile([P, CHT, cfg.KC], DT)
            nc.sync.dma_start(wconv_s[:], wconv_d.ap().rearrange(
                "p (c k) -> p c k", c=CHT))
            bconv_s = pp.tile([P, CHT], DT)
            nc.sync.dma_start(bconv_s[:], bconv_d.ap())
            wx_s, wdt_s, bdt_s, acol_s = {}, {}, {}, {}
            for d in "fb":
                wx_s[d] = pp.tile([P_CH, CHT, E], BF, name=f"wx{d}_s")
                nc.sync.dma_start(wx_s[d][:], wxT_d[d].ap().rearrange(
                    "(c p) e -> p c e", p=P_CH))
                wdt_s[d] = pp.tile([R, DC], BF, name=f"wdt{d}_s")
                nc.sync.dma_start(wdt_s[d][:], wdtT_d[d].ap())
                bdt_s[d] = pp.tile([P, CHT], DT, name=f"bdt{d}_s")
                nc.sync.dma_start(bdt_s[d][:], bdt_d[d].ap())
                acol_s[d] = pp.tile([P, NT], DT, name=f"acol{d}_s")
                nc.sync.dma_start(acol_s[d][:], acol_d[d].ap())
            dsum_s = pp.tile([P, CHT], DT)
            nc.sync.dma_start(dsum_s[:], dsum_d.ap())
            wout_s = pp.tile([P_CH, CHT, M], BF)
            nc.sync.dma_start(wout_s[:], woutT_d.ap().rearrange(
                "(c p) m -> p c m", p=P_CH))

            u_c = [pp.tile([P_CH, TOK], BF, name=f"u_c{c}") for c in range(CHT)]
            sres = [pp.tile([P_CH, TOK], BF, name=f"sres{c}")
                    for c in range(CHT)]

            # ---------- phase 0-2: x^T, in_proj, conv, silu ----------
            with tc.tile_pool(name="proj", bufs=1) as jp, \
                 tc.tile_pool(name="proj_ps", bufs=1, space="PSUM") as jpp:
                xT = [jp.tile([P, TOK], BF, name=f"xT{mt}") for mt in range(MT)]
                win_s = jp.tile([P, MT, 2 * DC], BF)
                nc.sync.dma_start(win_s[:, :, :DC], winuT_d.ap().rearrange(
                    "(a p) c -> p a c", p=P))
                nc.sync.dma_start(win_s[:, :, DC:], winrT_d.ap().rearrange(
                    "(a p) c -> p a c", p=P))

                TPG = min(4, MT)  # transposes grouped per PSUM tile
                for tb in range(TBT):
                    xsb = jp.tile([P, M], BF, tag="xsb", bufs=2, name="xsb")
                    nc.sync.dma_start(xsb[:], x_d.ap()[tb * P:(tb + 1) * P, :])
                    for mg in range(MT // TPG):
                        tp_ps = jpp.tile([P, TPG * P], BF, tag="tp", bufs=4,
                                         name="tp_ps")
                        for k in range(TPG):
                            mt = mg * TPG + k
                            nc.tensor.transpose(
                                tp_ps[:, k * P:(k + 1) * P],
                                xsb[:, mt * P:(mt + 1) * P], ident_s[:])
                        for k in range(TPG):
                            mt = mg * TPG + k
                            nc.vector.tensor_copy(
                                xT[mt][:, tb * P:(tb + 1) * P],
                                tp_ps[:, k * P:(k + 1) * P])

                # padded conv inputs (filled by in_proj PSUM evacuation)
                upad = [[jp.tile([P_CH, cfg.KC - 1 + L], BF,
                                 name=f"upad{c}_{b}")
                         for b in range(cfg.B)] for c in range(CHT)]
                for c in range(CHT):
                    for b in range(cfg.B):
                        nc.gpsimd.memset(upad[c][b][:, :cfg.KC - 1], 0.0)

                for c in range(CHT):
                    for fc in range(NFC):
                        f0 = fc * FCH
                        ups = jpp.tile([P_CH, FCH], DT, tag="mm", bufs=4,
                                       name="ups")
                        for kt in range(MT):
                            nc.tensor.matmul(
                                ups[:],
                                win_s[:, kt, c * P_CH:(c + 1) * P_CH],
                                xT[kt][:, f0:f0 + FCH],
                                start=(kt == 0), stop=(kt == MT - 1))
                        b = f0 // L
                        off = f0 % L
                        nc.scalar.copy(
                            upad[c][b][:, cfg.KC - 1 + off:
                                       cfg.KC - 1 + off + FCH], ups[:])

                # depthwise causal conv (tap products then tree add; all bf16
                # 4x-mode tensor_scalar/tensor_tensor) + single-pass SiLU
                with tc.tile_pool(name="conv", bufs=1) as cp:
                    for c in range(CHT):
                        for b in range(cfg.B):
                            tp_ = []
                            for k in range(cfg.KC):
                                tap = upad[c][b][:, k:k + L]
                                wk = wconv_s[:P_CH, c, k:k + 1]
                                t_ = cp.tile([P_CH, L], BF, tag=f"ct{k}",
                                             bufs=2, name=f"ct{k}")
                                if k == 0:
                                    nc.vector.tensor_scalar(
                                        t_[:], tap, wk,
                                        bconv_s[:P_CH, c:c + 1],
                                        OP.mult, OP.add)
                                else:
                                    nc.vector.tensor_scalar(
                                        t_[:], tap, wk, None, OP.mult)
                                tp_.append(t_)
                            s01 = cp.tile([P_CH, L], BF, tag="s01", bufs=2,
                                          name="s01")
                            nc.vector.tensor_tensor(s01[:], tp_[0][:],
                                                    tp_[1][:], OP.add)
                            s23 = cp.tile([P_CH, L], BF, tag="s23", bufs=2,
                                          name="s23")
                            nc.vector.tensor_tensor(s23[:], tp_[2][:],
                                                    tp_[3][:], OP.add)
                            acc = cp.tile([P_CH, L], BF, tag="cacc", bufs=2,
                                          name="cacc")
                            nc.vector.tensor_tensor(acc[:], s01[:], s23[:],
                                                    OP.add)
                            nc.scalar.activation(
                                u_c[c][:, b * L:(b + 1) * L], acc[:], AF.Silu)

                # ------ phase 3: dbc partials (bf16) + AllReduce; the res
                # projection is emitted between the two directions so it
                # overlaps the first AllReduce's network time ------
                dbc_part = {d: dp.tile([E, TOK], BF, name=f"dbc_part_{d}")
                            for d in "fb"}
                dbc_red = {d: dp.tile([E, TOK], BF, addr_space=cc_space,
                                      name=f"dbc_red_{d}") for d in "fb"}

                def dbc_dir(d):
                    for fc in range(NFC):
                        f0 = fc * FCH
                        bps = jpp.tile([E, FCH], DT, tag="mm", bufs=4,
                                       name="bps")
                        for c in range(CHT):
                            nc.tensor.matmul(
                                bps[:],
                                wx_s[d][:, c, :],
                                u_c[c][:, f0:f0 + FCH],
                                start=(c == 0), stop=(c == CHT - 1))
                        bst = jp.tile([E, FCH], BF, tag="bst", bufs=3,
                                      name="bst")
                        nc.scalar.copy(bst[:], bps[:])
                        nc.sync.dma_start(dbc_part[d][:, f0:f0 + FCH], bst[:])
                    nc.gpsimd.collective_compute(
                        "AllReduce", OP.add, replica_groups=rg,
                        ins=[dbc_part[d].opt()], outs=[dbc_red[d].opt()])

                dbc_dir("f")
                for c in range(CHT):
                    for fc in range(NFC):
                        f0 = fc * FCH
                        rps = jpp.tile([P_CH, FCH], DT, tag="mm", bufs=4,
                                       name="rps")
                        for kt in range(MT):
                            nc.tensor.matmul(
                                rps[:],
                                win_s[:, kt, DC + c * P_CH:DC + (c + 1) * P_CH],
                                xT[kt][:, f0:f0 + FCH],
                                start=(kt == 0), stop=(kt == MT - 1))
                        # sres = silu(res); the 0.5 factor is folded into
                        # W_out host-side
                        nc.scalar.activation(sres[c][:, f0:f0 + FCH], rps[:],
                                             AF.Silu)
                dbc_dir("b")
                if DBG:
                    nc.sync.dma_start(dbg["d_u0"].ap(), u_c[0][:])
                    nc.sync.dma_start(dbg["d_sres0"].ap(), sres[0][:])
                    nc.sync.dma_start(dbg["d_xT0"].ap(), xT[0][:])
                    for d2 in "fb":
                        nc.sync.dma_start(dbc_dbg[d2].ap(), dbc_red[d2][:])

            # ---------- phase 4: per-direction delta prep + scan ----------
            # Scan tiles are batch-merged [128, TOK]: one scan instruction
            # spans both batch segments; dA at each later segment's first
            # (in scan order) element is zeroed so no state leaks across.
            y_f = [pp.tile([P_CH, TOK], BF, name=f"y_f{c}") for c in range(CHT)]

            with tc.tile_pool(name="scan_sb", bufs=1) as sp, \
                 tc.tile_pool(name="scan_ps", bufs=1, space="PSUM") as spp, \
                 tc.tile_pool(name="comb", bufs=1) as kp:
                for d in "fb":
                    # dt/BC from the reduced projection
                    dt_sb = sp.tile([R, TOK], BF, tag="dt", bufs=1,
                                    name=f"dt_{d}")
                    nc.sync.dma_start(dt_sb[:], dbc_red[d][:R, :])
                    bc_sb = sp.tile([2 * N, TOK], BF, tag="bc", bufs=1,
                                    name=f"bc_{d}")
                    nc.sync.dma_start(bc_sb[:], dbc_red[d][R:, :])

                    # B/C replicated across the 8-channel groups, full TOK
                    brep = sp.tile([P, TOK], BF, tag="brep", bufs=2,
                                   name=f"brep{d}")
                    crep = sp.tile([P, TOK], BF, tag="crep", bufs=2,
                                   name=f"crep{d}")
                    for which, rep in ((0, brep), (1, crep)):
                        for lh in range(TOK // LH):
                            o = lh * LH
                            rps2 = spp.tile([P, LH], DT, tag="rep",
                                            bufs=2, name="rps2")
                            nc.tensor.matmul(
                                rps2[:],
                                tsel_s[:, which, :],
                                bc_sb[:, o:o + LH],
                                start=True, stop=True)
                            nc.scalar.copy(rep[:, o:o + LH], rps2[:])

                    # delta = softplus(dt @ WdtT + bdt) [bf16]; w = delta * u
                    delta = [sp.tile([P_CH, TOK], BF, tag=f"delta{c}", bufs=2,
                                     name=f"delta_{d}{c}") for c in range(CHT)]
                    w_s = [sp.tile([P_CH, TOK], BF, tag=f"w{c}", bufs=2,
                                   name=f"w_{d}{c}") for c in range(CHT)]
                    for c in range(CHT):
                        for fc in range(NFC):
                            f0 = fc * FCH
                            dps = spp.tile([P_CH, FCH], DT, tag="rep", bufs=2,
                                           name="dps")
                            nc.tensor.matmul(
                                dps[:],
                                wdt_s[d][:, c * P_CH:(c + 1) * P_CH],
                                dt_sb[:, f0:f0 + FCH],
                                start=True, stop=True)
                            # softplus(x + bdt) = ln(1 + exp(x + bdt))
                            spt = sp.tile([P_CH, FCH], BF, tag="spt", bufs=1,
                                          name="spt")
                            nc.scalar.activation(
                                spt[:], dps[:], AF.Exp,
                                bias=bdt_s[d][:P_CH, c:c + 1])
                            nc.scalar.activation(
                                delta[c][:, f0:f0 + FCH], spt[:], AF.Ln,
                                bias=1.0)
                        nc.vector.tensor_tensor(
                            w_s[c][:], delta[c][:], u_c[c][:], OP.mult)
                        if DBG and d == "f" and c == 0:
                            nc.sync.dma_start(dbg["d_delta0"].ap(),
                                              delta[0][:])

                    for j in range(NT):
                        c = j // TPC
                        jj = j % TPC
                        rsel = rall_s[:, jj, :]
                        dA = sp.tile([P, TOK], BF, tag="dA", bufs=2,
                                     name="dA")
                        dBu = sp.tile([P, TOK], BF, tag="dBu", bufs=2,
                                      name="dBu")
                        for b in range(cfg.B):
                            o = b * L
                            dp_ps = spp.tile([P, L], DT, tag="rep", bufs=2,
                                             name="dp_ps")
                            for lh in range(NLH):
                                q = lh * LH
                                nc.tensor.matmul(
                                    dp_ps[:, q:q + LH], rsel,
                                    delta[c][:, o + q:o + q + LH],
                                    start=True, stop=True)
                            nc.scalar.activation(
                                dA[:, o:o + L], dp_ps[:], AF.Exp,
                                scale=acol_s[d][:, j:j + 1])
                            w_ps = spp.tile([P, L], DT, tag="rep", bufs=2,
                                            name="w_ps")
                            for lh in range(NLH):
                                q = lh * LH
                                nc.tensor.matmul(
                                    w_ps[:, q:q + LH], rsel,
                                    w_s[c][:, o + q:o + q + LH],
                                    start=True, stop=True)
                            nc.vector.tensor_tensor(
                                dBu[:, o:o + L], w_ps[:],
                                brep[:, o:o + L], OP.mult)
                        # kill cross-batch state leakage at the segment
                        # boundary in scan order
                        if d == "f":
                            nc.gpsimd.memset(dA[:, L:L + 1], 0.0)
                        else:
                            nc.gpsimd.memset(dA[:, L - 1:L], 0.0)
                        h = sp.tile([P, TOK], BF, tag="h", bufs=2, name="h")
                        if d == "f":
                            nc.vector.tensor_tensor_scan(
                                h[:], dA[:], dBu[:], 0.0, OP.mult, OP.add)
                        else:
                            nc.vector.tensor_tensor_scan(
                                h[:, ::-1], dA[:, ::-1], dBu[:, ::-1],
                                0.0, OP.mult, OP.add)
                        if DBG and d == "f" and j == 0:
                            nc.sync.dma_start(dbg["d_dA0"].ap(), dA[:])
                            nc.sync.dma_start(dbg["d_dBu0"].ap(), dBu[:])
                            nc.sync.dma_start(dbg["d_h0"].ap(), h[:])
                        hC = sp.tile([P, TOK], BF, tag="hC", bufs=2,
                                     name="hC")
                        nc.vector.tensor_tensor(hC[:], h[:], crep[:], OP.mult)
                        if jj == 0:
                            y_ps = [spp.tile([P_CH, L], DT, tag=f"y{b}",
                                             bufs=1, name=f"y_ps{b}")
                                    for b in range(cfg.B)]
                        for b in range(cfg.B):
                            for lh in range(NLH):
                                q = lh * LH
                                nc.tensor.matmul(
                                    y_ps[b][:, q:q + LH],
                                    sall_s[:, jj, :],
                                    hC[:, b * L + q:b * L + q + LH],
                                    start=(jj == 0), stop=(jj == TPC - 1))
                        if jj != TPC - 1:
                            continue
                        for b in range(cfg.B):
                            ysl = y_f[c][:, b * L:(b + 1) * L]
                            if d == "f":
                                nc.scalar.copy(ysl, y_ps[b][:])
                            else:
                                # fused combine:
                                # y = (y_f + y_b + u*(fD+bD)) * (0.5*silu(res))
                                # (the 0.5 is folded into W_out host-side)
                                t1 = kp.tile([P_CH, L], BF, tag="t5", bufs=2,
                                             name="t1")
                                nc.vector.tensor_tensor(t1[:], y_ps[b][:],
                                                        ysl, OP.add)
                                t2 = kp.tile([P_CH, L], BF, tag="t5", bufs=2,
                                             name="t2")
                                nc.vector.scalar_tensor_tensor(
                                    t2[:], u_c[c][:, b * L:(b + 1) * L],
                                    dsum_s[:P_CH, c:c + 1], t1[:],
                                    OP.mult, OP.add)
                                nc.vector.tensor_tensor(
                                    ysl, t2[:], sres[c][:, b * L:(b + 1) * L],
                                    OP.mult)

            if DBG:
                nc.sync.dma_start(dbg["d_yf0"].ap(), y_f[0][:])
            # ---------- phase 6: out_proj + chunked ReduceScatter ----------
            # RS chunk k covers token rows [k*RCH, (k+1)*RCH); each core ends
            # with RSH rows per chunk, written to out_d rows [k*RSH,(k+1)*RSH).
            out_part = [dp.tile([RCH, M], DT, name=f"out_part{k}")
                        for k in range(RSC)]
            out_rs = [dp.tile([RSH, M], DT, name=f"out_rs{k}")
                      for k in range(RSC)]
            TBC = RCH // P  # token blocks per RS chunk
            with tc.tile_pool(name="out_ps", bufs=1, space="PSUM") as opp, \
                 tc.tile_pool(name="out_sb", bufs=1) as osp:
                MFC = min(512, M)
                for k in range(RSC):
                    for tbl in range(TBC):
                        tb = k * TBC + tbl
                        ops = opp.tile([P, M], DT, tag="out", bufs=2,
                                       name="ops")
                        for mc in range(M // MFC):
                            o = mc * MFC
                            for c in range(CHT):
                                nc.tensor.matmul(
                                    ops[:, o:o + MFC],
                                    y_f[c][:, tb * P:(tb + 1) * P],
                                    wout_s[:, c, o:o + MFC],
                                    start=(c == 0), stop=(c == CHT - 1))
                        ost = osp.tile([P, M], DT, tag="ost", bufs=2,
                                       name="ost")
                        nc.scalar.copy(ost[:], ops[:])
                        nc.sync.dma_start(
                            out_part[k][tbl * P:(tbl + 1) * P, :], ost[:])
                    nc.gpsimd.collective_compute(
                        "ReduceScatter", OP.add, replica_groups=rg,
                        ins=[out_part[k].opt()], outs=[out_rs[k].opt()])
                    nc.sync.dma_start(
                        out_d.ap()[k * RSH:(k + 1) * RSH, :], out_rs[k][:])

    nc.compile()
    return nc


# --------------------------------------------------------------------------
# host side
# --------------------------------------------------------------------------

def host_prep(cfg: Cfg, inputs: dict) -> list[dict]:
    """Slice the full-model inputs into one input map per core."""
    P = 128
    f32 = np.float32
    bf16 = ml_dtypes.bfloat16

    def g(name):
        return np.asarray(inputs[name], f32)

    x = g("x").reshape(cfg.TOK, cfg.M)
    W_in = g("W_in")
    W_conv = g("W_conv").reshape(cfg.DI, cfg.KC)
    b_conv = g("b_conv")
    W_out = g("W_out")
    ident, r_all, t_sel, s_all = build_consts(cfg)
    sall_flat = s_all.reshape(P, cfg.TPC * cfg.P_CH)
    rall_flat = r_all.reshape(cfg.P_CH, cfg.TPC * P)
    tsel_flat = t_sel.reshape(2 * cfg.N, 2 * P)

    per = {}
    for d in "fb":
        per[d] = dict(
            A=-np.exp(g(d + "A_log")),            # (DI, N)
            D=g(d + "D"),
            Wx=g(d + "Wx"),                       # (E, DI)
            Wdt=g(d + "Wdt"),                     # (DI, R)
            bdt=g(d + "bdt"),
        )

    def col_layout(v):  # (DC,) -> (P_CH, CHT): [p, c] = v[c*P_CH + p]
        return np.ascontiguousarray(
            v.reshape(cfg.CHT, cfg.P_CH).T.astype(f32))

    def pad_p(a):  # pad partition dim up to 128
        if a.shape[0] == P:
            return np.ascontiguousarray(a.astype(f32))
        out = np.zeros((P,) + a.shape[1:], f32)
        out[:a.shape[0]] = a
        return out

    in_maps = []
    for core in range(cfg.n_cores):
        c0 = core * cfg.DC
        ch = slice(c0, c0 + cfg.DC)
        m = {
            "x": x.astype(bf16),
            "winuT": np.ascontiguousarray(W_in[ch, :].T).astype(bf16),
            "winrT": np.ascontiguousarray(
                W_in[cfg.DI + c0:cfg.DI + c0 + cfg.DC, :].T).astype(bf16),
            "wconv": pad_p(
                W_conv[ch].reshape(cfg.CHT, cfg.P_CH, cfg.KC)
                .transpose(1, 0, 2).reshape(cfg.P_CH, cfg.CHT * cfg.KC)),
            "bconv": pad_p(col_layout(b_conv[ch])),
            "dsum": pad_p(col_layout(per["f"]["D"][ch] + per["b"]["D"][ch])),
            "woutT": np.ascontiguousarray(W_out[:, ch].T * 0.5).astype(bf16),
            "ident": ident.astype(bf16),
            "rall": rall_flat.astype(bf16),
            "tsel": tsel_flat.astype(bf16),
            "sall": sall_flat.astype(bf16),
        }
        for d in "fb":
            pd = per[d]
            m[f"wx{d}T"] = np.ascontiguousarray(pd["Wx"][:, ch].T).astype(bf16)
            m[f"wdt{d}T"] = np.ascontiguousarray(pd["Wdt"][ch, :].T).astype(bf16)
            m[f"bdt{d}"] = pad_p(col_layout(pd["bdt"][ch]))
            # A columns: [p, j] = A[8j + p//16, p%16] (local channels)
            Ac = pd["A"][ch]                       # (DC, N)
            acol = np.empty((P, cfg.NT), f32)
            pidx = np.arange(P)
            for j in range(cfg.NT):
                acol[:, j] = Ac[8 * j + pidx // 16, pidx % 16]
            m[f"acol{d}"] = acol
        in_maps.append({k: np.ascontiguousarray(v) for k, v in m.items()})
    return in_maps


def gather_out(cfg: Cfg, results: list[dict]) -> np.ndarray:
    """Reassemble chunked-ReduceScatter shards.

    Core c's out_rs rows [k*RSH, (k+1)*RSH) correspond to global token rows
    [k*RCH + c*RSH, k*RCH + (c+1)*RSH).
    """
    RCH = cfg.TOK // cfg.RSC
    RSH = RCH // cfg.n_cores
    out = np.empty((cfg.TOK, cfg.M), np.float32)
    for c in range(cfg.n_cores):
        shard = np.asarray(results[c]["out_rs"])
        for k in range(cfg.RSC):
            out[k * RCH + c * RSH:k * RCH + (c + 1) * RSH, :] = \
                shard[k * RSH:(k + 1) * RSH, :]
    return out.reshape(cfg.B, cfg.L, cfg.M).astype(np.float32)


def kernel(**inputs) -> np.ndarray:
    cfg = FULL
    from concourse.bass_utils import run_bass_kernel_spmd
    nc = build_program(cfg)
    in_maps = host_prep(cfg, inputs)
    res = run_bass_kernel_spmd(nc, in_maps, core_ids=list(range(cfg.n_cores)))
    return gather_out(cfg, res.results)


# revision 9
# speedup vs baseline: 1.3590x; 1.2009x over previous
"""Bidirectional Mamba block (in_proj -> depthwise causal conv -> SiLU ->
forward+backward S6 selective scan -> gated combine -> out_proj) as a
Trainium2 Bass/Tile SPMD kernel over 8 NeuronCores.

Sharding: tensor-parallel over d_inner (256 channels per core). The conv and
the S6 scans are channel-independent, so they need no communication. Two
small collectives:
  * AllReduce (bf16) of the partial x-projection dbc = u @ Wx^T per direction
  * Chunked ReduceScatter of the partial out-projection, overlapped with the
    out_proj matmuls; the host reassembles the 8 shards.

Compute dtypes: bf16 operands everywhere (fp32 PSUM accumulation), which
doubles/quadruples DVE elementwise throughput and halves DMA traffic. The S6
recurrence runs on the DVE tensor_tensor_scan (fp32 internal state).
Activation-table usage is phase-ordered (Silu early, Exp/Ln for the scan
phase) to avoid ACT_TABLE_LOAD thrash.
"""

import os
import sys

for _p in ("/opt/trn_rl_repo", "/root/.axon_site/_ro/trn_rl_repo"):
    if os.path.isdir(_p) and _p not in sys.path:
        sys.path.append(_p)

from dataclasses import dataclass

import ml_dtypes
import numpy as np

import concourse.bass as bass
import concourse.mybir as mybir
import concourse.tile as tile
from concourse import bacc

DT = mybir.dt.float32
BF = mybir.dt.bfloat16
AF = mybir.ActivationFunctionType
OP = mybir.AluOpType


@dataclass(frozen=True)
class Cfg:
    n_cores: int = 8
    B: int = 2
    L: int = 1024
    M: int = 1024      # d_model
    DI: int = 2048     # d_inner
    N: int = 16        # d_state
    R: int = 64        # dt_rank
    KC: int = 4        # conv kernel
    RSC: int = 4       # ReduceScatter chunks

    @property
    def DC(self):  # channels per core
        return self.DI // self.n_cores

    @property
    def TOK(self):
        return self.B * self.L

    @property
    def P_CH(self):  # partitions per channel tile
        return min(128, self.DC)

    @property
    def CHT(self):  # channel tiles per core
        return self.DC // self.P_CH

    @property
    def NT(self):  # scan tiles per (dir, batch): 8 channels each
        return self.DC // 8

    @property
    def TPC(self):  # scan tiles per channel tile
        return self.P_CH // 8

    @property
    def FCH(self):  # matmul moving-dim chunk over tokens (never spans batches)
        return min(512, self.L)

    @property
    def E(self):
        return self.R + 2 * self.N

    def check(self):
        assert self.DC % 8 == 0 and self.DC % self.P_CH == 0
        assert self.M % 128 == 0
        assert self.TOK % 128 == 0 and self.TOK % self.FCH == 0
        assert self.L % min(512, self.L) == 0
        assert self.N == 16
        assert self.TOK % (self.RSC * self.n_cores) == 0


FULL = Cfg()


def build_consts(cfg: Cfg):
    """Selection matrices used as PE 'weights' (exact 0/1 values)."""
    P = 128
    ident = np.eye(P, dtype=np.float32)
    # R_all[:, jj, :]: out[p] = src[8*jj + p//16]  (delta/w replication)
    r_all = np.zeros((cfg.P_CH, cfg.TPC, P), np.float32)
    for jj in range(cfg.TPC):
        for p in range(P):
            r_all[8 * jj + p // 16, jj, p] = 1.0
    # T_sel[:, which, :]: out[p] = src[16*which + p%16]  (B/C replication)
    t_sel = np.zeros((2 * cfg.N, 2, P), np.float32)
    for which in range(2):
        for p in range(P):
            t_sel[cfg.N * which + p % 16, which, p] = 1.0
    # S_all[:, jj, :]: reduce groups of 16 partitions into channel 8*jj+p//16
    s_all = np.zeros((P, cfg.TPC, cfg.P_CH), np.float32)
    for jj in range(cfg.TPC):
        for p in range(P):
            s_all[p, jj, 8 * jj + p // 16] = 1.0
    return ident, r_all, t_sel, s_all


def build_program(cfg: Cfg) -> bass.Bass:
    cfg.check()
    P = 128
    TOK, L, M = cfg.TOK, cfg.L, cfg.M
    DC, CHT, P_CH, NT, TPC, FCH = (cfg.DC, cfg.CHT, cfg.P_CH, cfg.NT,
                                   cfg.TPC, cfg.FCH)
    MT = M // P               # m tiles
    TBT = TOK // P            # token blocks
    NFC = TOK // FCH          # token chunks
    E, R, N = cfg.E, cfg.R, cfg.N
    LH = min(512, L)          # matmul chunk within one sequence
    NLH = L // LH

    nc = bacc.Bacc(
        "TRN2", target_bir_lowering=False, debug=False, num_devices=cfg.n_cores
    )

    # ---- kernel I/O ----
    x_d = nc.dram_tensor("x", [TOK, M], BF, kind="ExternalInput")
    winuT_d = nc.dram_tensor("winuT", [M, DC], BF, kind="ExternalInput")
    winrT_d = nc.dram_tensor("winrT", [M, DC], BF, kind="ExternalInput")
    wconv_d = nc.dram_tensor("wconv", [P, CHT * cfg.KC], DT, kind="ExternalInput")
    bconv_d = nc.dram_tensor("bconv", [P, CHT], DT, kind="ExternalInput")
    wxT_d = {d: nc.dram_tensor(f"wx{d}T", [DC, E], BF, kind="ExternalInput")
             for d in "fb"}
    wdtT_d = {d: nc.dram_tensor(f"wdt{d}T", [R, DC], BF, kind="ExternalInput")
              for d in "fb"}
    bdt_d = {d: nc.dram_tensor(f"bdt{d}", [P, CHT], DT, kind="ExternalInput")
             for d in "fb"}
    acol_d = {d: nc.dram_tensor(f"acol{d}", [P_CH, CHT * N], DT,
                                kind="ExternalInput")
              for d in "fb"}
    dsum_d = nc.dram_tensor("dsum", [P, CHT], DT, kind="ExternalInput")
    woutT_d = nc.dram_tensor("woutT", [DC, M], BF, kind="ExternalInput")
    ident_d = nc.dram_tensor("ident", [P, P], BF, kind="ExternalInput")

    RSC = cfg.RSC
    RCH = TOK // RSC                    # rows per RS chunk
    RSH = RCH // cfg.n_cores            # rows per core per RS chunk
    out_d = nc.dram_tensor("out_rs", [TOK // cfg.n_cores, M], DT,
                           kind="ExternalOutput")
    DBG = os.environ.get("KDBG", "") == "1"
    dbg = {}
    if DBG:
        for nm, shp, dt_ in [("d_u0", [P_CH, TOK], BF),
                             ("d_sres0", [P_CH, TOK], BF),
                             ("d_delta0", [P_CH, TOK], BF),
                             ("d_dA0", [P, TOK], BF),
                             ("d_dBu0", [P, TOK], BF),
                             ("d_h0", [P, TOK], BF),
                             ("d_yf0", [P_CH, TOK], BF),
                             ("d_xT0", [P, TOK], BF)]:
            dbg[nm] = nc.dram_tensor(nm, shp, dt_, kind="ExternalOutput")
    dbc_dbg = {d: nc.dram_tensor(f"d_dbc_{d}", [E, TOK], BF,
                                 kind="ExternalOutput") if DBG else None
               for d in "fb"}

    rg = [list(range(cfg.n_cores))]
    cc_space = "Shared" if cfg.n_cores > 4 else "Local"

    with tile.TileContext(nc) as tc:
        with tc.tile_pool(name="persist", bufs=1) as pp, \
             tc.tile_pool(name="dram", bufs=1, space="DRAM") as dp:

            # ---------- persistent SBUF (small weights + gate activations) --
            ident_s = pp.tile([P, P], BF)
            nc.sync.dma_start(ident_s[:], ident_d.ap())
            wconv_s = pp.tile([P, CHT, cfg.KC], DT)
            nc.sync.dma_start(wconv_s[:], wconv_d.ap().rearrange(
                "p (c k) -> p c k", c=CHT))
            bconv_s = pp.tile([P, CHT], DT)
            nc.sync.dma_start(bconv_s[:], bconv_d.ap())
            wx_s, wdt_s, bdt_s, acol_s = {}, {}, {}, {}
            for d in "fb":
                wx_s[d] = pp.tile([P_CH, CHT, E], BF, name=f"wx{d}_s")
                nc.sync.dma_start(wx_s[d][:], wxT_d[d].ap().rearrange(
                    "(c p) e -> p c e", p=P_CH))
                wdt_s[d] = pp.tile([R, DC], BF, name=f"wdt{d}_s")
                nc.sync.dma_start(wdt_s[d][:], wdtT_d[d].ap())
                bdt_s[d] = pp.tile([P, CHT], DT, name=f"bdt{d}_s")
                nc.sync.dma_start(bdt_s[d][:], bdt_d[d].ap())
                acol_s[d] = pp.tile([P_CH, CHT, N], DT, name=f"acol{d}_s")
                nc.sync.dma_start(acol_s[d][:], acol_d[d].ap().rearrange(
                    "p (c n) -> p c n", c=CHT))
            dsum_s = pp.tile([P, CHT], DT)
            nc.sync.dma_start(dsum_s[:], dsum_d.ap())
            wout_s = pp.tile([P_CH, CHT, M], BF)
            nc.sync.dma_start(wout_s[:], woutT_d.ap().rearrange(
                "(c p) m -> p c m", p=P_CH))

            u_c = [pp.tile([P_CH, TOK], BF, name=f"u_c{c}") for c in range(CHT)]
            sres = [pp.tile([P_CH, TOK], BF, name=f"sres{c}")
                    for c in range(CHT)]

            # ---------- phase 0-2: x^T, in_proj, conv, silu ----------
            with tc.tile_pool(name="proj", bufs=1) as jp, \
                 tc.tile_pool(name="proj_ps", bufs=1, space="PSUM") as jpp:
                xT = [jp.tile([P, TOK], BF, name=f"xT{mt}") for mt in range(MT)]
                win_s = jp.tile([P, MT, 2 * DC], BF)
                nc.sync.dma_start(win_s[:, :, :DC], winuT_d.ap().rearrange(
                    "(a p) c -> p a c", p=P))
                nc.sync.dma_start(win_s[:, :, DC:], winrT_d.ap().rearrange(
                    "(a p) c -> p a c", p=P))

                TPG = min(4, MT)  # transposes grouped per PSUM tile
                for tb in range(TBT):
                    xsb = jp.tile([P, M], BF, tag="xsb", bufs=2, name="xsb")
                    nc.sync.dma_start(xsb[:], x_d.ap()[tb * P:(tb + 1) * P, :])
                    for mg in range(MT // TPG):
                        tp_ps = jpp.tile([P, TPG * P], BF, tag="tp", bufs=4,
                                         name="tp_ps")
                        for k in range(TPG):
                            mt = mg * TPG + k
                            nc.tensor.transpose(
                                tp_ps[:, k * P:(k + 1) * P],
                                xsb[:, mt * P:(mt + 1) * P], ident_s[:])
                        for k in range(TPG):
                            mt = mg * TPG + k
                            nc.vector.tensor_copy(
                                xT[mt][:, tb * P:(tb + 1) * P],
                                tp_ps[:, k * P:(k + 1) * P])

                # padded conv inputs (filled by in_proj PSUM evacuation)
                upad = [[jp.tile([P_CH, cfg.KC - 1 + L], BF,
                                 name=f"upad{c}_{b}")
                         for b in range(cfg.B)] for c in range(CHT)]
                for c in range(CHT):
                    for b in range(cfg.B):
                        nc.gpsimd.memset(upad[c][b][:, :cfg.KC - 1], 0.0)

                for c in range(CHT):
                    for fc in range(NFC):
                        f0 = fc * FCH
                        ups = jpp.tile([P_CH, FCH], DT, tag="mm", bufs=4,
                                       name="ups")
                        for kt in range(MT):
                            nc.tensor.matmul(
                                ups[:],
                                win_s[:, kt, c * P_CH:(c + 1) * P_CH],
                                xT[kt][:, f0:f0 + FCH],
                                start=(kt == 0), stop=(kt == MT - 1))
                        b = f0 // L
                        off = f0 % L
                        nc.scalar.copy(
                            upad[c][b][:, cfg.KC - 1 + off:
                                       cfg.KC - 1 + off + FCH], ups[:])

                # depthwise causal conv (tap products then tree add; all bf16
                # 4x-mode tensor_scalar/tensor_tensor) + single-pass SiLU
                with tc.tile_pool(name="conv", bufs=1) as cp:
                    for c in range(CHT):
                        for b in range(cfg.B):
                            tp_ = []
                            for k in range(cfg.KC):
                                tap = upad[c][b][:, k:k + L]
                                wk = wconv_s[:P_CH, c, k:k + 1]
                                t_ = cp.tile([P_CH, L], BF, tag=f"ct{k}",
                                             bufs=2, name=f"ct{k}")
                                if k == 0:
                                    nc.vector.tensor_scalar(
                                        t_[:], tap, wk,
                                        bconv_s[:P_CH, c:c + 1],
                                        OP.mult, OP.add)
                                else:
                                    nc.vector.tensor_scalar(
                                        t_[:], tap, wk, None, OP.mult)
                                tp_.append(t_)
                            s01 = cp.tile([P_CH, L], BF, tag="s01", bufs=2,
                                          name="s01")
                            nc.vector.tensor_tensor(s01[:], tp_[0][:],
                                                    tp_[1][:], OP.add)
                            s23 = cp.tile([P_CH, L], BF, tag="s23", bufs=2,
                                          name="s23")
                            nc.vector.tensor_tensor(s23[:], tp_[2][:],
                                                    tp_[3][:], OP.add)
                            acc = cp.tile([P_CH, L], BF, tag="cacc", bufs=2,
                                          name="cacc")
                            nc.vector.tensor_tensor(acc[:], s01[:], s23[:],
                                                    OP.add)
                            nc.scalar.activation(
                                u_c[c][:, b * L:(b + 1) * L], acc[:], AF.Silu)

                # ------ phase 3: dbc partials (bf16) + AllReduce; the res
                # projection is emitted between the two directions so it
                # overlaps the first AllReduce's network time ------
                dbc_part = {d: dp.tile([E, TOK], BF, name=f"dbc_part_{d}")
                            for d in "fb"}
                dbc_red = {d: dp.tile([E, TOK], BF, addr_space=cc_space,
                                      name=f"dbc_red_{d}") for d in "fb"}

                def dbc_dir(d):
                    for fc in range(NFC):
                        f0 = fc * FCH
                        bps = jpp.tile([E, FCH], DT, tag="mm", bufs=4,
                                       name="bps")
                        for c in range(CHT):
                            nc.tensor.matmul(
                                bps[:],
                                wx_s[d][:, c, :],
                                u_c[c][:, f0:f0 + FCH],
                                start=(c == 0), stop=(c == CHT - 1))
                        bst = jp.tile([E, FCH], BF, tag="bst", bufs=3,
                                      name="bst")
                        nc.scalar.copy(bst[:], bps[:])
                        nc.sync.dma_start(dbc_part[d][:, f0:f0 + FCH], bst[:])
                    nc.gpsimd.collective_compute(
                        "AllReduce", OP.add, replica_groups=rg,
                        ins=[dbc_part[d].opt()], outs=[dbc_red[d].opt()])

                dbc_dir("f")
                for c in range(CHT):
                    for fc in range(NFC):
                        f0 = fc * FCH
                        rps = jpp.tile([P_CH, FCH], DT, tag="mm", bufs=4,
                                       name="rps")
                        for kt in range(MT):
                            nc.tensor.matmul(
                                rps[:],
                                win_s[:, kt, DC + c * P_CH:DC + (c + 1) * P_CH],
                                xT[kt][:, f0:f0 + FCH],
                                start=(kt == 0), stop=(kt == MT - 1))
                        # sres = silu(res); the 0.5 factor is folded into
                        # W_out host-side
                        nc.scalar.activation(sres[c][:, f0:f0 + FCH], rps[:],
                                             AF.Silu)
                dbc_dir("b")
                if DBG:
                    nc.sync.dma_start(dbg["d_u0"].ap(), u_c[0][:])
                    nc.sync.dma_start(dbg["d_sres0"].ap(), sres[0][:])
                    nc.sync.dma_start(dbg["d_xT0"].ap(), xT[0][:])
                    for d2 in "fb":
                        nc.sync.dma_start(dbc_dbg[d2].ap(), dbc_red[d2][:])

            # ---------- phase 4: per-direction delta prep + scan ----------
            # Channel-partition layout: each scan tile is [128 channels,
            # G*N states x TOK tokens] with the state index in the FREE dim.
            # B/C rows are broadcast across partitions straight from the
            # AllReduce result in DRAM (contiguous per-partition reads), so
            # dA/dBu/hC are pure SBUF bf16 elementwise ops and the only PE
            # work left is the identity-accumulate that sums hC over states.
            # One scan instruction spans G states x B batches; dA is zeroed
            # at every segment's first element (in scan order) so no state
            # leaks across segment boundaries.
            y_f = [pp.tile([P_CH, TOK], BF, name=f"y_f{c}") for c in range(CHT)]
            G = 2                      # states per scan group
            NG = N // G                # groups per (dir, channel tile)

            with tc.tile_pool(name="scan_sb", bufs=1) as sp, \
                 tc.tile_pool(name="scan_ps", bufs=1, space="PSUM") as spp, \
                 tc.tile_pool(name="comb", bufs=1) as kp:
                for d in "fb":
                    # dt rows of the reduced projection
                    dt_sb = sp.tile([R, TOK], BF, tag="dt", bufs=1,
                                    name=f"dt_{d}")
                    nc.sync.dma_start(dt_sb[:], dbc_red[d][:R, :])

                    # delta = softplus(dt @ WdtT + bdt) [bf16]; w = delta * u
                    # (Exp and Ln batched separately to avoid ACT-table
                    # reload thrash.)
                    delta = [sp.tile([P_CH, TOK], BF, tag=f"delta{c}", bufs=2,
                                     name=f"delta_{d}{c}") for c in range(CHT)]
                    w_s = [sp.tile([P_CH, TOK], BF, tag=f"w{c}", bufs=2,
                                   name=f"w_{d}{c}") for c in range(CHT)]
                    spt = [sp.tile([P_CH, TOK], BF, tag=f"spt{c}", bufs=2,
                                   name=f"spt{c}") for c in range(CHT)]
                    for c in range(CHT):
                        for fc in range(NFC):
                            f0 = fc * FCH
                            dps = spp.tile([P_CH, FCH], DT, tag="rep", bufs=2,
                                           name="dps")
                            nc.tensor.matmul(
                                dps[:],
                                wdt_s[d][:, c * P_CH:(c + 1) * P_CH],
                                dt_sb[:, f0:f0 + FCH],
                                start=True, stop=True)
                            # softplus(x + bdt) = ln(1 + exp(x + bdt))
                            nc.scalar.activation(
                                spt[c][:, f0:f0 + FCH], dps[:], AF.Exp,
                                bias=bdt_s[d][:P_CH, c:c + 1])
                    for c in range(CHT):
                        nc.scalar.activation(delta[c][:], spt[c][:], AF.Ln,
                                             bias=1.0)
                        nc.vector.tensor_tensor(
                            w_s[c][:], delta[c][:], u_c[c][:], OP.mult)
                        if DBG and d == "f" and c == 0:
                            nc.sync.dma_start(dbg["d_delta0"].ap(),
                                              delta[0][:])

                    for c in range(CHT):
                        y_ps = spp.tile([P_CH, TOK], DT, tag="y", bufs=1,
                                        name="y_ps")
                        for g in range(NG):
                            n0 = g * G
                            # B/C state rows broadcast to all 128 partitions
                            # straight from DRAM (contiguous 2-row reads).
                            Bg = sp.tile([P, G, TOK], BF, tag="Bg", bufs=2,
                                         name="Bg")
                            nc.sync.dma_start(
                                Bg[:],
                                dbc_red[d][R + n0:R + n0 + G, :]
                                .unsqueeze(0).broadcast_to([P, G, TOK]))
                            Cg = sp.tile([P, G, TOK], BF, tag="Cg", bufs=2,
                                         name="Cg")
                            nc.sync.dma_start(
                                Cg[:],
                                dbc_red[d][R + N + n0:R + N + n0 + G, :]
                                .unsqueeze(0).broadcast_to([P, G, TOK]))

                            dA = sp.tile([P_CH, G, TOK], BF, tag="dA", bufs=2,
                                         name="dA")
                            for i in range(G):
                                nc.scalar.activation(
                                    dA[:, i, :], delta[c][:], AF.Exp,
                                    scale=acol_s[d][:P_CH, c, n0 + i:n0 + i + 1])
                            dBu = sp.tile([P_CH, G, TOK], BF, tag="dBu",
                                          bufs=2, name="dBu")
                            nc.vector.tensor_tensor(
                                dBu[:],
                                w_s[c][:].unsqueeze(1)
                                .broadcast_to([P_CH, G, TOK]),
                                Bg[:], OP.mult)
                            # zero dA at every segment start (scan order) so
                            # state never crosses (state, batch) boundaries;
                            # zeroing the very first scan element is harmless
                            # because the scan initial is 0.
                            flat = dA[:].rearrange("p a b -> p (a b)")
                            if d == "f":
                                nc.gpsimd.memset(flat[:, 0::L], 0.0)
                            else:
                                nc.gpsimd.memset(flat[:, L - 1::L], 0.0)
                            h = sp.tile([P_CH, G, TOK], BF, tag="h", bufs=2,
                                        name="h")
                            hf = h[:].rearrange("p a b -> p (a b)")
                            dAf = dA[:].rearrange("p a b -> p (a b)")
                            dBuf = dBu[:].rearrange("p a b -> p (a b)")
                            if d == "f":
                                nc.vector.tensor_tensor_scan(
                                    hf, dAf, dBuf, 0.0, OP.mult, OP.add)
                            else:
                                nc.vector.tensor_tensor_scan(
                                    hf[:, ::-1], dAf[:, ::-1], dBuf[:, ::-1],
                                    0.0, OP.mult, OP.add)
                            if DBG and d == "f" and c == 0 and g == 0:
                                nc.sync.dma_start(dbg["d_dA0"].ap(),
                                                  dAf[:, :TOK])
                                nc.sync.dma_start(dbg["d_dBu0"].ap(),
                                                  dBuf[:, :TOK])
                                nc.sync.dma_start(dbg["d_h0"].ap(),
                                                  hf[:, :TOK])
                            hC = sp.tile([P_CH, G, TOK], BF, tag="hC", bufs=2,
                                         name="hC")
                            nc.vector.tensor_tensor(hC[:], h[:], Cg[:],
                                                    OP.mult)
                            # y += hC summed over the G states (identity
                            # accumulate on the tensor engine)
                            for i in range(G):
                                for lh in range(TOK // LH):
                                    q = lh * LH
                                    nc.tensor.matmul(
                                        y_ps[:, q:q + LH],
                                        ident_s[:],
                                        hC[:, i, q:q + LH],
                                        start=(g == 0 and i == 0),
                                        stop=(g == NG - 1 and i == G - 1))
                        # evacuate / combine
                        if d == "f":
                            nc.scalar.copy(y_f[c][:], y_ps[:])
                        else:
                            # y = (y_f + y_b + u*(fD+bD)) * (0.5*silu(res))
                            # (the 0.5 is folded into W_out host-side)
                            t1 = kp.tile([P_CH, TOK], BF, tag="t5", bufs=2,
                                         name="t1")
                            nc.vector.tensor_tensor(t1[:], y_ps[:],
                                                    y_f[c][:], OP.add)
                            t2 = kp.tile([P_CH, TOK], BF, tag="t5", bufs=2,
                                         name="t2")
                            nc.vector.scalar_tensor_tensor(
                                t2[:], u_c[c][:],
                                dsum_s[:P_CH, c:c + 1], t1[:],
                                OP.mult, OP.add)
                            nc.vector.tensor_tensor(
                                y_f[c][:], t2[:], sres[c][:], OP.mult)

            if DBG:
                nc.sync.dma_start(dbg["d_yf0"].ap(), y_f[0][:])
            # ---------- phase 6: out_proj + chunked ReduceScatter ----------
            # RS chunk k covers token rows [k*RCH, (k+1)*RCH); each core ends
            # with RSH rows per chunk, written to out_d rows [k*RSH,(k+1)*RSH).
            out_part = [dp.tile([RCH, M], DT, name=f"out_part{k}")
                        for k in range(RSC)]
            out_rs = [dp.tile([RSH, M], DT, name=f"out_rs{k}")
                      for k in range(RSC)]
            TBC = RCH // P  # token blocks per RS chunk
            with tc.tile_pool(name="out_ps", bufs=1, space="PSUM") as opp, \
                 tc.tile_pool(name="out_sb", bufs=1) as osp:
                MFC = min(512, M)
                for k in range(RSC):
                    for tbl in range(TBC):
                        tb = k * TBC + tbl
                        ops = opp.tile([P, M], DT, tag="out", bufs=2,
                                       name="ops")
                        for mc in range(M // MFC):
                            o = mc * MFC
                            for c in range(CHT):
                                nc.tensor.matmul(
                                    ops[:, o:o + MFC],
                                    y_f[c][:, tb * P:(tb + 1) * P],
                                    wout_s[:, c, o:o + MFC],
                                    start=(c == 0), stop=(c == CHT - 1))
                        ost = osp.tile([P, M], DT, tag="ost", bufs=2,
                                       name="ost")
                        nc.scalar.copy(ost[:], ops[:])
                        nc.sync.dma_start(
                            out_part[k][tbl * P:(tbl + 1) * P, :], ost[:])
                    nc.gpsimd.collective_compute(
                        "ReduceScatter", OP.add, replica_groups=rg,
                        ins=[out_part[k].opt()], outs=[out_rs[k].opt()])
                    nc.sync.dma_start(
                        out_d.ap()[k * RSH:(k + 1) * RSH, :], out_rs[k][:])

    nc.compile()
    return nc


# --------------------------------------------------------------------------
# host side
# --------------------------------------------------------------------------

def host_prep(cfg: Cfg, inputs: dict) -> list[dict]:
    """Slice the full-model inputs into one input map per core."""
    P = 128
    f32 = np.float32
    bf16 = ml_dtypes.bfloat16

    def g(name):
        return np.asarray(inputs[name], f32)

    x = g("x").reshape(cfg.TOK, cfg.M)
    W_in = g("W_in")
    W_conv = g("W_conv").reshape(cfg.DI, cfg.KC)
    b_conv = g("b_conv")
    W_out = g("W_out")
    ident = np.eye(P, dtype=np.float32)

    per = {}
    for d in "fb":
        per[d] = dict(
            A=-np.exp(g(d + "A_log")),            # (DI, N)
            D=g(d + "D"),
            Wx=g(d + "Wx"),                       # (E, DI)
            Wdt=g(d + "Wdt"),                     # (DI, R)
            bdt=g(d + "bdt"),
        )

    def col_layout(v):  # (DC,) -> (P_CH, CHT): [p, c] = v[c*P_CH + p]
        return np.ascontiguousarray(
            v.reshape(cfg.CHT, cfg.P_CH).T.astype(f32))

    def pad_p(a):  # pad partition dim up to 128
        if a.shape[0] == P:
            return np.ascontiguousarray(a.astype(f32))
        out = np.zeros((P,) + a.shape[1:], f32)
        out[:a.shape[0]] = a
        return out

    in_maps = []
    for core in range(cfg.n_cores):
        c0 = core * cfg.DC
        ch = slice(c0, c0 + cfg.DC)
        m = {
            "x": x.astype(bf16),
            "winuT": np.ascontiguousarray(W_in[ch, :].T).astype(bf16),
            "winrT": np.ascontiguousarray(
                W_in[cfg.DI + c0:cfg.DI + c0 + cfg.DC, :].T).astype(bf16),
            "wconv": pad_p(
                W_conv[ch].reshape(cfg.CHT, cfg.P_CH, cfg.KC)
                .transpose(1, 0, 2).reshape(cfg.P_CH, cfg.CHT * cfg.KC)),
            "bconv": pad_p(col_layout(b_conv[ch])),
            "dsum": pad_p(col_layout(per["f"]["D"][ch] + per["b"]["D"][ch])),
            "woutT": np.ascontiguousarray(W_out[:, ch].T * 0.5).astype(bf16),
            "ident": ident.astype(bf16),
        }
        for d in "fb":
            pd = per[d]
            m[f"wx{d}T"] = np.ascontiguousarray(pd["Wx"][:, ch].T).astype(bf16)
            m[f"wdt{d}T"] = np.ascontiguousarray(pd["Wdt"][ch, :].T).astype(bf16)
            m[f"bdt{d}"] = pad_p(col_layout(pd["bdt"][ch]))
            # A columns: [p, (c, n)] = A[c*P_CH + p, n] (local channels)
            Ac = pd["A"][ch]                       # (DC, N)
            m[f"acol{d}"] = np.ascontiguousarray(
                Ac.reshape(cfg.CHT, cfg.P_CH, cfg.N)
                .transpose(1, 0, 2).reshape(cfg.P_CH, cfg.CHT * cfg.N)
                .astype(f32))
        in_maps.append({k: np.ascontiguousarray(v) for k, v in m.items()})
    return in_maps


def gather_out(cfg: Cfg, results: list[dict]) -> np.ndarray:
    """Reassemble chunked-ReduceScatter shards.

    Core c's out_rs rows [k*RSH, (k+1)*RSH) correspond to global token rows
    [k*RCH + c*RSH, k*RCH + (c+1)*RSH).
    """
    RCH = cfg.TOK // cfg.RSC
    RSH = RCH // cfg.n_cores
    out = np.empty((cfg.TOK, cfg.M), np.float32)
    for c in range(cfg.n_cores):
        shard = np.asarray(results[c]["out_rs"])
        for k in range(cfg.RSC):
            out[k * RCH + c * RSH:k * RCH + (c + 1) * RSH, :] = \
                shard[k * RSH:(k + 1) * RSH, :]
    return out.reshape(cfg.B, cfg.L, cfg.M).astype(np.float32)


def kernel(**inputs) -> np.ndarray:
    cfg = FULL
    from concourse.bass_utils import run_bass_kernel_spmd
    nc = build_program(cfg)
    in_maps = host_prep(cfg, inputs)
    res = run_bass_kernel_spmd(nc, in_maps, core_ids=list(range(cfg.n_cores)))
    return gather_out(cfg, res.results)


# revision 12
# speedup vs baseline: 1.3757x; 1.0123x over previous
"""Bidirectional Mamba block (in_proj -> depthwise causal conv -> SiLU ->
forward+backward S6 selective scan -> gated combine -> out_proj) as a
Trainium2 Bass/Tile SPMD kernel over 8 NeuronCores.

Sharding: tensor-parallel over d_inner (256 channels per core). The conv and
the S6 scans are channel-independent, so they need no communication. Two
small collectives:
  * AllReduce (bf16) of the partial x-projection dbc = u @ Wx^T per direction
  * Chunked ReduceScatter of the partial out-projection, overlapped with the
    out_proj matmuls; the host reassembles the 8 shards.

Compute dtypes: bf16 operands everywhere (fp32 PSUM accumulation), which
doubles/quadruples DVE elementwise throughput and halves DMA traffic. The S6
recurrence runs on the DVE tensor_tensor_scan (fp32 internal state).
Activation-table usage is phase-ordered (Silu early, Exp/Ln for the scan
phase) to avoid ACT_TABLE_LOAD thrash.
"""

import os
import sys

for _p in ("/opt/trn_rl_repo", "/root/.axon_site/_ro/trn_rl_repo"):
    if os.path.isdir(_p) and _p not in sys.path:
        sys.path.append(_p)

from dataclasses import dataclass

import ml_dtypes
import numpy as np

import concourse.bass as bass
import concourse.mybir as mybir
import concourse.tile as tile
from concourse import bacc

DT = mybir.dt.float32
BF = mybir.dt.bfloat16
AF = mybir.ActivationFunctionType
OP = mybir.AluOpType


@dataclass(frozen=True)
class Cfg:
    n_cores: int = 8
    B: int = 2
    L: int = 1024
    M: int = 1024      # d_model
    DI: int = 2048     # d_inner
    N: int = 16        # d_state
    R: int = 64        # dt_rank
    KC: int = 4        # conv kernel
    RSC: int = 4       # ReduceScatter chunks

    @property
    def DC(self):  # channels per core
        return self.DI // self.n_cores

    @property
    def TOK(self):
        return self.B * self.L

    @property
    def P_CH(self):  # partitions per channel tile
        return min(128, self.DC)

    @property
    def CHT(self):  # channel tiles per core
        return self.DC // self.P_CH

    @property
    def NT(self):  # scan tiles per (dir, batch): 8 channels each
        return self.DC // 8

    @property
    def TPC(self):  # scan tiles per channel tile
        return self.P_CH // 8

    @property
    def FCH(self):  # matmul moving-dim chunk over tokens (never spans batches)
        return min(512, self.L)

    @property
    def E(self):
        return self.R + 2 * self.N

    def check(self):
        assert self.DC % 8 == 0 and self.DC % self.P_CH == 0
        assert self.M % 128 == 0
        assert self.TOK % 128 == 0 and self.TOK % self.FCH == 0
        assert self.L % min(512, self.L) == 0
        assert self.N == 16
        assert self.TOK % (self.RSC * self.n_cores) == 0


FULL = Cfg()


def build_consts(cfg: Cfg):
    """Selection matrices used as PE 'weights' (exact 0/1 values)."""
    P = 128
    ident = np.eye(P, dtype=np.float32)
    # R_all[:, jj, :]: out[p] = src[8*jj + p//16]  (delta/w replication)
    r_all = np.zeros((cfg.P_CH, cfg.TPC, P), np.float32)
    for jj in range(cfg.TPC):
        for p in range(P):
            r_all[8 * jj + p // 16, jj, p] = 1.0
    # T_sel[:, which, :]: out[p] = src[16*which + p%16]  (B/C replication)
    t_sel = np.zeros((2 * cfg.N, 2, P), np.float32)
    for which in range(2):
        for p in range(P):
            t_sel[cfg.N * which + p % 16, which, p] = 1.0
    # S_all[:, jj, :]: reduce groups of 16 partitions into channel 8*jj+p//16
    s_all = np.zeros((P, cfg.TPC, cfg.P_CH), np.float32)
    for jj in range(cfg.TPC):
        for p in range(P):
            s_all[p, jj, 8 * jj + p // 16] = 1.0
    return ident, r_all, t_sel, s_all


def build_program(cfg: Cfg) -> bass.Bass:
    cfg.check()
    P = 128
    TOK, L, M = cfg.TOK, cfg.L, cfg.M
    DC, CHT, P_CH, NT, TPC, FCH = (cfg.DC, cfg.CHT, cfg.P_CH, cfg.NT,
                                   cfg.TPC, cfg.FCH)
    MT = M // P               # m tiles
    TBT = TOK // P            # token blocks
    NFC = TOK // FCH          # token chunks
    E, R, N = cfg.E, cfg.R, cfg.N
    LH = min(512, L)          # matmul chunk within one sequence
    NLH = L // LH

    nc = bacc.Bacc(
        "TRN2", target_bir_lowering=False, debug=False, num_devices=cfg.n_cores
    )

    # ---- kernel I/O ----
    x_d = nc.dram_tensor("x", [TOK, M], BF, kind="ExternalInput")
    winuT_d = nc.dram_tensor("winuT", [M, DC], BF, kind="ExternalInput")
    winrT_d = nc.dram_tensor("winrT", [M, DC], BF, kind="ExternalInput")
    wconv_d = nc.dram_tensor("wconv", [P, CHT * cfg.KC], DT, kind="ExternalInput")
    bconv_d = nc.dram_tensor("bconv", [P, CHT], DT, kind="ExternalInput")
    wxT_d = {d: nc.dram_tensor(f"wx{d}T", [DC, E], BF, kind="ExternalInput")
             for d in "fb"}
    wdtT_d = {d: nc.dram_tensor(f"wdt{d}T", [R, DC], BF, kind="ExternalInput")
              for d in "fb"}
    bdt_d = {d: nc.dram_tensor(f"bdt{d}", [P, CHT], DT, kind="ExternalInput")
             for d in "fb"}
    acol_d = {d: nc.dram_tensor(f"acol{d}", [P_CH, CHT * N], DT,
                                kind="ExternalInput")
              for d in "fb"}
    dsum_d = nc.dram_tensor("dsum", [P, CHT], DT, kind="ExternalInput")
    woutT_d = nc.dram_tensor("woutT", [DC, M], BF, kind="ExternalInput")
    ident_d = nc.dram_tensor("ident", [P, P], BF, kind="ExternalInput")

    RSC = cfg.RSC
    RCH = TOK // RSC                    # rows per RS chunk
    RSH = RCH // cfg.n_cores            # rows per core per RS chunk
    out_d = nc.dram_tensor("out_rs", [TOK // cfg.n_cores, M], BF,
                           kind="ExternalOutput")
    DBG = os.environ.get("KDBG", "") == "1"
    dbg = {}
    if DBG:
        for nm, shp, dt_ in [("d_u0", [P_CH, TOK], BF),
                             ("d_sres0", [P_CH, TOK], BF),
                             ("d_delta0", [P_CH, TOK], BF),
                             ("d_dA0", [P, TOK], BF),
                             ("d_dBu0", [P, TOK], BF),
                             ("d_h0", [P, TOK], BF),
                             ("d_yf0", [P_CH, TOK], BF),
                             ("d_xT0", [P, TOK], BF)]:
            dbg[nm] = nc.dram_tensor(nm, shp, dt_, kind="ExternalOutput")
    dbc_dbg = {d: nc.dram_tensor(f"d_dbc_{d}", [E, TOK], BF,
                                 kind="ExternalOutput") if DBG else None
               for d in "fb"}

    rg = [list(range(cfg.n_cores))]
    cc_space = "Shared" if cfg.n_cores > 4 else "Local"

    with tile.TileContext(nc) as tc:
        with tc.tile_pool(name="persist", bufs=1) as pp, \
             tc.tile_pool(name="dram", bufs=1, space="DRAM") as dp:

            # ---------- persistent SBUF (small weights + gate activations) --
            ident_s = pp.tile([P, P], BF)
            nc.sync.dma_start(ident_s[:], ident_d.ap())
            wconv_s = pp.tile([P, CHT, cfg.KC], DT)
            nc.sync.dma_start(wconv_s[:], wconv_d.ap().rearrange(
                "p (c k) -> p c k", c=CHT))
            bconv_s = pp.tile([P, CHT], DT)
            nc.sync.dma_start(bconv_s[:], bconv_d.ap())
            wx_s, wdt_s, bdt_s, acol_s = {}, {}, {}, {}
            for d in "fb":
                wx_s[d] = pp.tile([P_CH, CHT, E], BF, name=f"wx{d}_s")
                nc.sync.dma_start(wx_s[d][:], wxT_d[d].ap().rearrange(
                    "(c p) e -> p c e", p=P_CH))
                wdt_s[d] = pp.tile([R, DC], BF, name=f"wdt{d}_s")
                nc.sync.dma_start(wdt_s[d][:], wdtT_d[d].ap())
                bdt_s[d] = pp.tile([P, CHT], DT, name=f"bdt{d}_s")
                nc.sync.dma_start(bdt_s[d][:], bdt_d[d].ap())
                acol_s[d] = pp.tile([P_CH, CHT, N], DT, name=f"acol{d}_s")
                nc.sync.dma_start(acol_s[d][:], acol_d[d].ap().rearrange(
                    "p (c n) -> p c n", c=CHT))
            dsum_s = pp.tile([P, CHT], DT)
            nc.sync.dma_start(dsum_s[:], dsum_d.ap())
            wout_s = pp.tile([P_CH, CHT, M], BF)
            nc.sync.dma_start(wout_s[:], woutT_d.ap().rearrange(
                "(c p) m -> p c m", p=P_CH))

            u_c = [pp.tile([P_CH, TOK], BF, name=f"u_c{c}") for c in range(CHT)]
            sres = [pp.tile([P_CH, TOK], BF, name=f"sres{c}")
                    for c in range(CHT)]
            uD = [pp.tile([P_CH, TOK], BF, name=f"uD{c}") for c in range(CHT)]

            # ---------- phase 0-2: x^T, in_proj, conv, silu ----------
            with tc.tile_pool(name="proj", bufs=1) as jp, \
                 tc.tile_pool(name="proj_ps", bufs=1, space="PSUM") as jpp:
                xT = [jp.tile([P, TOK], BF, name=f"xT{mt}") for mt in range(MT)]
                win_s = jp.tile([P, MT, 2 * DC], BF)
                nc.sync.dma_start(win_s[:, :, :DC], winuT_d.ap().rearrange(
                    "(a p) c -> p a c", p=P))
                nc.sync.dma_start(win_s[:, :, DC:], winrT_d.ap().rearrange(
                    "(a p) c -> p a c", p=P))

                TPG = min(4, MT)  # transposes grouped per PSUM tile
                for tb in range(TBT):
                    xsb = jp.tile([P, M], BF, tag="xsb", bufs=2, name="xsb")
                    nc.sync.dma_start(xsb[:], x_d.ap()[tb * P:(tb + 1) * P, :])
                    for mg in range(MT // TPG):
                        tp_ps = jpp.tile([P, TPG * P], BF, tag="tp", bufs=4,
                                         name="tp_ps")
                        for k in range(TPG):
                            mt = mg * TPG + k
                            nc.tensor.transpose(
                                tp_ps[:, k * P:(k + 1) * P],
                                xsb[:, mt * P:(mt + 1) * P], ident_s[:])
                        for k in range(TPG):
                            mt = mg * TPG + k
                            nc.vector.tensor_copy(
                                xT[mt][:, tb * P:(tb + 1) * P],
                                tp_ps[:, k * P:(k + 1) * P])

                # padded conv inputs (filled by in_proj PSUM evacuation)
                upad = [[jp.tile([P_CH, cfg.KC - 1 + L], BF,
                                 name=f"upad{c}_{b}")
                         for b in range(cfg.B)] for c in range(CHT)]
                for c in range(CHT):
                    for b in range(cfg.B):
                        nc.gpsimd.memset(upad[c][b][:, :cfg.KC - 1], 0.0)

                for c in range(CHT):
                    for fc in range(NFC):
                        f0 = fc * FCH
                        ups = jpp.tile([P_CH, FCH], DT, tag="mm", bufs=4,
                                       name="ups")
                        for kt in range(MT):
                            nc.tensor.matmul(
                                ups[:],
                                win_s[:, kt, c * P_CH:(c + 1) * P_CH],
                                xT[kt][:, f0:f0 + FCH],
                                start=(kt == 0), stop=(kt == MT - 1))
                        b = f0 // L
                        off = f0 % L
                        nc.scalar.copy(
                            upad[c][b][:, cfg.KC - 1 + off:
                                       cfg.KC - 1 + off + FCH], ups[:])

                # depthwise causal conv (tap products then tree add; all bf16
                # 4x-mode tensor_scalar/tensor_tensor) + single-pass SiLU
                with tc.tile_pool(name="conv", bufs=1) as cp:
                    for c in range(CHT):
                        for b in range(cfg.B):
                            tp_ = []
                            for k in range(cfg.KC):
                                tap = upad[c][b][:, k:k + L]
                                wk = wconv_s[:P_CH, c, k:k + 1]
                                t_ = cp.tile([P_CH, L], BF, tag=f"ct{k}",
                                             bufs=2, name=f"ct{k}")
                                if k == 0:
                                    nc.vector.tensor_scalar(
                                        t_[:], tap, wk,
                                        bconv_s[:P_CH, c:c + 1],
                                        OP.mult, OP.add)
                                else:
                                    nc.vector.tensor_scalar(
                                        t_[:], tap, wk, None, OP.mult)
                                tp_.append(t_)
                            s01 = cp.tile([P_CH, L], BF, tag="s01", bufs=2,
                                          name="s01")
                            nc.vector.tensor_tensor(s01[:], tp_[0][:],
                                                    tp_[1][:], OP.add)
                            s23 = cp.tile([P_CH, L], BF, tag="s23", bufs=2,
                                          name="s23")
                            nc.vector.tensor_tensor(s23[:], tp_[2][:],
                                                    tp_[3][:], OP.add)
                            acc = cp.tile([P_CH, L], BF, tag="cacc", bufs=2,
                                          name="cacc")
                            nc.vector.tensor_tensor(acc[:], s01[:], s23[:],
                                                    OP.add)
                            nc.scalar.activation(
                                u_c[c][:, b * L:(b + 1) * L], acc[:], AF.Silu)

                # ------ phase 3: dbc partials (bf16) + AllReduce; the res
                # projection is emitted between the two directions so it
                # overlaps the first AllReduce's network time ------
                dbc_part = {d: dp.tile([E, TOK], BF, name=f"dbc_part_{d}")
                            for d in "fb"}
                dt_part = {d: dp.tile([R, TOK], BF, name=f"dt_part_{d}")
                           for d in "fb"}
                dbc_red = {d: dp.tile([E, TOK], BF, addr_space=cc_space,
                                      name=f"dbc_red_{d}") for d in "fb"}
                dt_red = {d: dp.tile([R, TOK], BF, addr_space=cc_space,
                                     name=f"dt_red_{d}") for d in "fb"}

                def dbc_dir(d):
                    for fc in range(NFC):
                        f0 = fc * FCH
                        bps = jpp.tile([E, FCH], DT, tag="mm", bufs=4,
                                       name="bps")
                        for c in range(CHT):
                            nc.tensor.matmul(
                                bps[:],
                                wx_s[d][:, c, :],
                                u_c[c][:, f0:f0 + FCH],
                                start=(c == 0), stop=(c == CHT - 1))
                        bst = jp.tile([E, FCH], BF, tag="bst", bufs=3,
                                      name="bst")
                        nc.scalar.copy(bst[:], bps[:])
                        nc.sync.dma_start(dt_part[d][:, f0:f0 + FCH],
                                          bst[:R, :])
                        nc.sync.dma_start(dbc_part[d][R:, f0:f0 + FCH],
                                          bst[R:, :])
                    nc.gpsimd.collective_compute(
                        "AllReduce", OP.add, replica_groups=rg,
                        ins=[dt_part[d].opt()], outs=[dt_red[d].opt()])
                    nc.gpsimd.collective_compute(
                        "AllReduce", OP.add, replica_groups=rg,
                        ins=[dbc_part[d][R:, :].opt()],
                        outs=[dbc_red[d][R:, :].opt()])

                for c in range(CHT):
                    nc.vector.tensor_scalar(
                        uD[c][:], u_c[c][:], dsum_s[:P_CH, c:c + 1], None,
                        OP.mult)
                dbc_dir("f")
                for c in range(CHT):
                    for fc in range(NFC):
                        f0 = fc * FCH
                        rps = jpp.tile([P_CH, FCH], DT, tag="mm", bufs=4,
                                       name="rps")
                        for kt in range(MT):
                            nc.tensor.matmul(
                                rps[:],
                                win_s[:, kt, DC + c * P_CH:DC + (c + 1) * P_CH],
                                xT[kt][:, f0:f0 + FCH],
                                start=(kt == 0), stop=(kt == MT - 1))
                        # sres = silu(res); the 0.5 factor is folded into
                        # W_out host-side
                        nc.scalar.activation(sres[c][:, f0:f0 + FCH], rps[:],
                                             AF.Silu)
                dbc_dir("b")
                if DBG:
                    nc.sync.dma_start(dbg["d_u0"].ap(), u_c[0][:])
                    nc.sync.dma_start(dbg["d_sres0"].ap(), sres[0][:])
                    nc.sync.dma_start(dbg["d_xT0"].ap(), xT[0][:])
                    for d2 in "fb":
                        nc.sync.dma_start(dbc_dbg[d2].ap(), dbc_red[d2][:])

            # ---------- phase 4: per-direction delta prep + scan ----------
            # Channel-partition layout: each scan tile is [128 channels,
            # G*N states x TOK tokens] with the state index in the FREE dim.
            # B/C rows are broadcast across partitions straight from the
            # AllReduce result in DRAM (contiguous per-partition reads), so
            # dA/dBu/hC are pure SBUF bf16 elementwise ops and the only PE
            # work left is the identity-accumulate that sums hC over states.
            # One scan instruction spans G states x B batches; dA is zeroed
            # at every segment's first element (in scan order) so no state
            # leaks across segment boundaries.
            y_f = [pp.tile([P_CH, TOK], BF, name=f"y_f{c}") for c in range(CHT)]
            G = 2                      # states per scan group
            NG = N // G                # groups per (dir, channel tile)

            with tc.tile_pool(name="scan_sb", bufs=1) as sp, \
                 tc.tile_pool(name="scan_ps", bufs=1, space="PSUM") as spp, \
                 tc.tile_pool(name="comb", bufs=1) as kp:
                for d in "fb":
                    # dt rows of the reduced projection
                    dt_sb = sp.tile([R, TOK], BF, tag="dt", bufs=1,
                                    name=f"dt_{d}")
                    nc.sync.dma_start(dt_sb[:], dt_red[d][:])

                    # delta = softplus(dt @ WdtT + bdt) [bf16]; w = delta * u
                    # (Exp and Ln batched separately to avoid ACT-table
                    # reload thrash.)
                    delta = [sp.tile([P_CH, TOK], BF, tag=f"delta{c}", bufs=2,
                                     name=f"delta_{d}{c}") for c in range(CHT)]
                    spt = [sp.tile([P_CH, TOK], BF, tag=f"spt{c}", bufs=1,
                                   name=f"spt{c}") for c in range(CHT)]
                    for c in range(CHT):
                        for fc in range(NFC):
                            f0 = fc * FCH
                            dps = spp.tile([P_CH, FCH], DT, tag="rep", bufs=2,
                                           name="dps")
                            nc.tensor.matmul(
                                dps[:],
                                wdt_s[d][:, c * P_CH:(c + 1) * P_CH],
                                dt_sb[:, f0:f0 + FCH],
                                start=True, stop=True)
                            # softplus(x + bdt) = ln(1 + exp(x + bdt))
                            nc.scalar.activation(
                                spt[c][:, f0:f0 + FCH], dps[:], AF.Exp,
                                bias=bdt_s[d][:P_CH, c:c + 1])
                    w2 = [sp.tile([P_CH, G, TOK], BF, tag=f"w2_{c}",
                                  bufs=2, name=f"w2_{d}{c}")
                          for c in range(CHT)]
                    for c in range(CHT):
                        nc.scalar.activation(delta[c][:], spt[c][:], AF.Ln,
                                             bias=1.0)
                        nc.vector.tensor_tensor(
                            w2[c][:],
                            delta[c][:].unsqueeze(1)
                            .broadcast_to([P_CH, G, TOK]),
                            u_c[c][:].unsqueeze(1)
                            .broadcast_to([P_CH, G, TOK]), OP.mult)
                        if DBG and d == "f" and c == 0:
                            nc.sync.dma_start(dbg["d_delta0"].ap(),
                                              delta[0][:])

                    for c in range(CHT):
                        y_ps = spp.tile([P_CH, TOK], DT, tag="y", bufs=1,
                                        name="y_ps")
                        for g in range(NG):
                            n0 = g * G
                            # B/C state rows broadcast to all 128 partitions
                            # straight from DRAM (contiguous 2-row reads).
                            Bg = sp.tile([P, G, TOK], BF, tag="Bg", bufs=2,
                                         name="Bg")
                            nc.sync.dma_start(
                                Bg[:],
                                dbc_red[d][R + n0:R + n0 + G, :]
                                .unsqueeze(0).broadcast_to([P, G, TOK]))
                            Cg = sp.tile([P, G, TOK], BF, tag="Cg", bufs=2,
                                         name="Cg")
                            nc.sync.dma_start(
                                Cg[:],
                                dbc_red[d][R + N + n0:R + N + n0 + G, :]
                                .unsqueeze(0).broadcast_to([P, G, TOK]))

                            dA = sp.tile([P_CH, G, TOK], BF, tag="dA", bufs=2,
                                         name="dA")
                            for i in range(G):
                                nc.scalar.activation(
                                    dA[:, i, :], delta[c][:], AF.Exp,
                                    scale=acol_s[d][:P_CH, c, n0 + i:n0 + i + 1])
                            dBu = sp.tile([P_CH, G, TOK], BF, tag="dBu",
                                          bufs=2, name="dBu")
                            nc.vector.tensor_tensor(
                                dBu[:].rearrange("p a b -> p (a b)"),
                                w2[c][:].rearrange("p a b -> p (a b)"),
                                Bg[:].rearrange("p a b -> p (a b)"), OP.mult)
                            # zero dA at every segment start (scan order) so
                            # state never crosses (state, batch) boundaries;
                            # zeroing the very first scan element is harmless
                            # because the scan initial is 0.
                            flat = dA[:].rearrange("p a b -> p (a b)")
                            if d == "f":
                                nc.gpsimd.memset(flat[:, 0::L], 0.0)
                            else:
                                nc.gpsimd.memset(flat[:, L - 1::L], 0.0)
                            h = sp.tile([P_CH, G, TOK], BF, tag="h", bufs=2,
                                        name="h")
                            hf = h[:].rearrange("p a b -> p (a b)")
                            dAf = dA[:].rearrange("p a b -> p (a b)")
                            dBuf = dBu[:].rearrange("p a b -> p (a b)")
                            if d == "f":
                                nc.vector.tensor_tensor_scan(
                                    hf, dAf, dBuf, 0.0, OP.mult, OP.add)
                            else:
                                nc.vector.tensor_tensor_scan(
                                    hf[:, ::-1], dAf[:, ::-1], dBuf[:, ::-1],
                                    0.0, OP.mult, OP.add)
                            if DBG and d == "f" and c == 0 and g == 0:
                                nc.sync.dma_start(dbg["d_dA0"].ap(),
                                                  dAf[:, :TOK])
                                nc.sync.dma_start(dbg["d_dBu0"].ap(),
                                                  dBuf[:, :TOK])
                                nc.sync.dma_start(dbg["d_h0"].ap(),
                                                  hf[:, :TOK])
                            hC = sp.tile([P_CH, G, TOK], BF, tag="hC", bufs=2,
                                         name="hC")
                            nc.vector.tensor_tensor(
                                hC[:].rearrange("p a b -> p (a b)"),
                                h[:].rearrange("p a b -> p (a b)"),
                                Cg[:].rearrange("p a b -> p (a b)"), OP.mult)
                            # y += hC summed over the G states (identity
                            # accumulate on the tensor engine)
                            for i in range(G):
                                for lh in range(TOK // LH):
                                    q = lh * LH
                                    nc.tensor.matmul(
                                        y_ps[:, q:q + LH],
                                        ident_s[:],
                                        hC[:, i, q:q + LH],
                                        start=(g == 0 and i == 0),
                                        stop=(g == NG - 1 and i == G - 1))
                        # evacuate / combine (all-SBUF bf16 ops)
                        if d == "f":
                            nc.scalar.copy(y_f[c][:], y_ps[:])
                        else:
                            # y = (y_f + y_b + u*(fD+bD)) * (0.5*silu(res))
                            # (the 0.5 is folded into W_out host-side)
                            yb = kp.tile([P_CH, TOK], BF, tag="t5", bufs=2,
                                         name="yb")
                            nc.scalar.copy(yb[:], y_ps[:])
                            t1 = kp.tile([P_CH, TOK], BF, tag="t5", bufs=2,
                                         name="t1")
                            nc.vector.tensor_tensor(t1[:], yb[:],
                                                    y_f[c][:], OP.add)
                            t2 = kp.tile([P_CH, TOK], BF, tag="t5", bufs=2,
                                         name="t2")
                            nc.vector.tensor_tensor(t2[:], t1[:], uD[c][:],
                                                    OP.add)
                            nc.vector.tensor_tensor(
                                y_f[c][:], t2[:], sres[c][:], OP.mult)

            if DBG:
                nc.sync.dma_start(dbg["d_yf0"].ap(), y_f[0][:])
            # ---------- phase 6: out_proj + chunked ReduceScatter ----------
            # RS chunk k covers token rows [k*RCH, (k+1)*RCH); each core ends
            # with RSH rows per chunk, written to out_d rows [k*RSH,(k+1)*RSH).
            out_part = [dp.tile([RCH, M], BF, name=f"out_part{k}")
                        for k in range(RSC)]
            out_rs = [dp.tile([RSH, M], BF, name=f"out_rs{k}")
                      for k in range(RSC)]
            TBC = RCH // P  # token blocks per RS chunk
            with tc.tile_pool(name="out_ps", bufs=1, space="PSUM") as opp, \
                 tc.tile_pool(name="out_sb", bufs=1) as osp:
                MFC = min(512, M)
                for k in range(RSC):
                    for tbl in range(TBC):
                        tb = k * TBC + tbl
                        ops = opp.tile([P, M], DT, tag="out", bufs=2,
                                       name="ops")
                        for mc in range(M // MFC):
                            o = mc * MFC
                            for c in range(CHT):
                                nc.tensor.matmul(
                                    ops[:, o:o + MFC],
                                    y_f[c][:, tb * P:(tb + 1) * P],
                                    wout_s[:, c, o:o + MFC],
                                    start=(c == 0), stop=(c == CHT - 1))
                        ost = osp.tile([P, M], BF, tag="ost", bufs=2,
                                       name="ost")
                        nc.scalar.copy(ost[:], ops[:])
                        nc.sync.dma_start(
                            out_part[k][tbl * P:(tbl + 1) * P, :], ost[:])
                    nc.gpsimd.collective_compute(
                        "ReduceScatter", OP.add, replica_groups=rg,
                        ins=[out_part[k].opt()], outs=[out_rs[k].opt()])
                    nc.sync.dma_start(
                        out_d.ap()[k * RSH:(k + 1) * RSH, :], out_rs[k][:])

    nc.compile()
    return nc


# --------------------------------------------------------------------------
# host side
# --------------------------------------------------------------------------

def host_prep(cfg: Cfg, inputs: dict) -> list[dict]:
    """Slice the full-model inputs into one input map per core."""
    P = 128
    f32 = np.float32
    bf16 = ml_dtypes.bfloat16

    def g(name):
        return np.asarray(inputs[name], f32)

    x = g("x").reshape(cfg.TOK, cfg.M)
    W_in = g("W_in")
    W_conv = g("W_conv").reshape(cfg.DI, cfg.KC)
    b_conv = g("b_conv")
    W_out = g("W_out")
    ident = np.eye(P, dtype=np.float32)

    per = {}
    for d in "fb":
        per[d] = dict(
            A=-np.exp(g(d + "A_log")),            # (DI, N)
            D=g(d + "D"),
            Wx=g(d + "Wx"),                       # (E, DI)
            Wdt=g(d + "Wdt"),                     # (DI, R)
            bdt=g(d + "bdt"),
        )

    def col_layout(v):  # (DC,) -> (P_CH, CHT): [p, c] = v[c*P_CH + p]
        return np.ascontiguousarray(
            v.reshape(cfg.CHT, cfg.P_CH).T.astype(f32))

    def pad_p(a):  # pad partition dim up to 128
        if a.shape[0] == P:
            return np.ascontiguousarray(a.astype(f32))
        out = np.zeros((P,) + a.shape[1:], f32)
        out[:a.shape[0]] = a
        return out

    in_maps = []
    for core in range(cfg.n_cores):
        c0 = core * cfg.DC
        ch = slice(c0, c0 + cfg.DC)
        m = {
            "x": x.astype(bf16),
            "winuT": np.ascontiguousarray(W_in[ch, :].T).astype(bf16),
            "winrT": np.ascontiguousarray(
                W_in[cfg.DI + c0:cfg.DI + c0 + cfg.DC, :].T).astype(bf16),
            "wconv": pad_p(
                W_conv[ch].reshape(cfg.CHT, cfg.P_CH, cfg.KC)
                .transpose(1, 0, 2).reshape(cfg.P_CH, cfg.CHT * cfg.KC)),
            "bconv": pad_p(col_layout(b_conv[ch])),
            "dsum": pad_p(col_layout(per["f"]["D"][ch] + per["b"]["D"][ch])),
            "woutT": np.ascontiguousarray(W_out[:, ch].T * 0.5).astype(bf16),
            "ident": ident.astype(bf16),
        }
        for d in "fb":
            pd = per[d]
            m[f"wx{d}T"] = np.ascontiguousarray(pd["Wx"][:, ch].T).astype(bf16)
            m[f"wdt{d}T"] = np.ascontiguousarray(pd["Wdt"][ch, :].T).astype(bf16)
            m[f"bdt{d}"] = pad_p(col_layout(pd["bdt"][ch]))
            # A columns: [p, (c, n)] = A[c*P_CH + p, n] (local channels)
            Ac = pd["A"][ch]                       # (DC, N)
            m[f"acol{d}"] = np.ascontiguousarray(
                Ac.reshape(cfg.CHT, cfg.P_CH, cfg.N)
                .transpose(1, 0, 2).reshape(cfg.P_CH, cfg.CHT * cfg.N)
                .astype(f32))
        in_maps.append({k: np.ascontiguousarray(v) for k, v in m.items()})
    return in_maps


def gather_out(cfg: Cfg, results: list[dict]) -> np.ndarray:
    """Reassemble chunked-ReduceScatter shards.

    Core c's out_rs rows [k*RSH, (k+1)*RSH) correspond to global token rows
    [k*RCH + c*RSH, k*RCH + (c+1)*RSH).
    """
    RCH = cfg.TOK // cfg.RSC
    RSH = RCH // cfg.n_cores
    out = np.empty((cfg.TOK, cfg.M), np.float32)
    for c in range(cfg.n_cores):
        shard = np.asarray(results[c]["out_rs"]).astype(np.float32)
        for k in range(cfg.RSC):
            out[k * RCH + c * RSH:k * RCH + (c + 1) * RSH, :] = \
                shard[k * RSH:(k + 1) * RSH, :]
    return out.reshape(cfg.B, cfg.L, cfg.M).astype(np.float32)


def kernel(**inputs) -> np.ndarray:
    cfg = FULL
    from concourse.bass_utils import run_bass_kernel_spmd
    nc = build_program(cfg)
    in_maps = host_prep(cfg, inputs)
    res = run_bass_kernel_spmd(nc, in_maps, core_ids=list(range(cfg.n_cores)))
    return gather_out(cfg, res.results)
